# revision 11
# baseline (speedup 1.0000x reference)
"""Trainium2 Bass kernel for a 2-hidden-layer LIF spiking network.

Math (per timestep t, per layer):
    v = 0.9*y + cur ;  spike s = (v >= 1) ;  y = v*(1-s) = v*u  with u = (v < 1)
Layer currents:
    cur1 = x_t @ W_ih            (x binary, precomputable for ALL t)
    cur2 = s1 @ W_hh = colsum(W_hh) - u1 @ W_hh
    cur3 = s2 @ W_ho = colsum(W_ho) - u2 @ W_ho
Output: rate = mean_t s_out = 1 - sum_t(u_out)/T

Key restructurings:
  * Layer 1's recurrence does not depend on layer 2, so all three matmuls are
    batched over the full (T*B) column space; only the cheap elementwise LIF
    scans are sequential in t.
  * Weights are quantized to 23-bit fixed point (step 2^-23) and decomposed
    into ND=6 exact signed base-16 digit planes, each stored in fp8 e5m2
    (digits in [-8,7] and power-of-2 scales are exact in e5m2). The moving
    operands (x and the spike complements u) carry the value 2^-14, exactly
    representable as the e5m2 minimum normal. Pairs of digit planes feed
    fp8 DoubleRow matmuls (2 stationary planes per instruction at 0.5
    cycles/row), so full 23-bit weight precision streams at 1.5 cycles/row
    vs 2.0 for an fp16 hi/lo pair -- with every product exact in fp32 PSUM.
  * The moving AP broadcasts the same spike tile across the DoubleRow pair
    (middle dim stride 0), so spikes are stored once, in SBUF, uncompressed
    (1 byte): the layer-1 spike complement for ALL timesteps stays resident
    and is never spilled to DRAM.

Sharding: data-parallel over batch (256/8 = 32 rows per core), weights
replicated, no cross-core communication.

Per-core schedule:
  Phase A (W_ih digit planes resident, 12.6MB, loaded progressively per
    output chunk so block 0 starts ~2.5us in): mm1 over blocks of 10 steps,
    LIF1 scan fused per block (v computed in-place over cur1), u1 written
    straight into its resident SBUF tile.
  Phase B (W_hh digit planes streamed from DRAM per 128-col output chunk,
    double-buffered): superblocks of 20 steps (last one split 2x10 so the
    final scan hides under mm2); mm2 -> cur2 with colsum correction fused
    into the PSUM->SBUF Identity-activation copy (scale=-1, bias=colsum),
    LIF2 scan, mm3 (emitted mid-way through the next superblock's mm2 so
    the PE never waits on the DVE scan), output LIF scan, final rate.
"""

import numpy as np

# ---- problem constants (hardcoded; kernel.py must be self-contained) ----
BATCH = 256
INPUT_DIM = 1024
HIDDEN_DIM = 2048
OUTPUT_DIM = 10
T = 100
NCORES = 8
BLOC = BATCH // NCORES          # 32 batch rows per core
TB = 5                          # timesteps per phase-A block
NBLK = T // TB                  # 20 blocks
CA = TB * BLOC                  # 160 columns per phase-A block
C = 320                         # columns per phase-B matmul chunk
COLS = T * BLOC                 # 3200 total columns
# phase-B superblocks as (col0, ncols): last 640 split so the tail overlaps
SUPERS = [(0, 640), (640, 640), (1280, 640), (1920, 640),
          (2560, 320), (2880, 320)]
SCMAX = 640
KI = INPUT_DIM // 128           # 8 k-chunks for mm1
KH = HIDDEN_DIM // 128          # 16 k-chunks (and m-chunks) for mm2
DECAY = 0.9
THRESH = 1.0
TH_NUDGE = 0.0                  # tie-break re-roll knob (harmless ~1e-6 scale)

ND = 6                          # digit planes (23-bit fixed point)
KBITS = 23                      # weight step 2^-KBITS (max digit range 7829367)
MOV = 2.0 ** -14                # moving-operand value (e5m2 min normal)
DMAX = 7 * (16 ** ND - 1) // 15

_BUILT = None


def _build():
    """Trace + compile the Bass program once."""
    from contextlib import ExitStack

    import concourse.bacc as bacc
    import concourse.tile as tile
    from concourse import mybir
    from concourse.alu_op_type import AluOpType as op

    f32 = mybir.dt.float32
    e5 = mybir.dt.float8e5
    DR = mybir.MatmulPerfMode.DoubleRow
    ident = mybir.ActivationFunctionType.Identity
    TH = THRESH + TH_NUDGE

    nc = bacc.Bacc("TRN2", target_bir_lowering=False, debug=False,
                   num_devices=NCORES)

    # x values {0, 2^-14}: [input_dim, t*b] t-major columns
    x_d = nc.dram_tensor("x", [INPUT_DIM, COLS], e5,
                         kind="ExternalInput").ap()
    # wih digit planes, m-chunk major: [(m*KI + kt)*128, dig*128]
    wih_d = nc.dram_tensor("wihd", [KH * INPUT_DIM, ND * 128], e5,
                           kind="ExternalInput").ap()
    # whh digit planes: [(m2*KH + kt)*128, dig*128]
    whh_d = nc.dram_tensor("whhd", [KH * HIDDEN_DIM, ND * 128], e5,
                           kind="ExternalInput").ap()
    # who planes padded to 16 cols: [kt*128, dig*16]
    who_d = nc.dram_tensor("whod", [KH * 128, ND * 16], e5,
                           kind="ExternalInput").ap()
    cs_hh_d = nc.dram_tensor("cs_hh", [128, KH], f32, kind="ExternalInput").ap()
    cs_ho_d = nc.dram_tensor("cs_ho", [OUTPUT_DIM, 1], f32,
                             kind="ExternalInput").ap()
    out_d = nc.dram_tensor("out", [OUTPUT_DIM, BLOC], f32,
                           kind="ExternalOutput").ap()

    with tile.TileContext(nc) as tc, ExitStack() as ctx:
        # u1 complement {0, 2^-14}: resident across both phases [p, kt, col]
        u1_pool = ctx.enter_context(tc.tile_pool(name="u1", bufs=1))
        u1 = u1_pool.tile([128, KH * COLS], e5, tag="u1")
        u1_3 = u1[:].rearrange("p (k c) -> p k c", k=KH)

        # ---------------- Phase A: mm1 + LIF1 scan ----------------
        with tc.tile_pool(name="wih", bufs=1) as wih_pool, \
             tc.tile_pool(name="xin", bufs=2) as x_pool, \
             tc.tile_pool(name="cur1", bufs=2) as cur1_pool, \
             tc.tile_pool(name="st1", bufs=1) as st1_pool, \
             tc.tile_pool(name="psA", bufs=6, space="PSUM") as psA:

            # x for block 0 first so the DMA queue starts useful work early
            x_first = x_pool.tile([128, KI * CA], e5, tag="x")
            xf_3 = x_first[:].rearrange("p (k c) -> p k c", k=KI)
            for k in range(KI):
                nc.sync.dma_start(xf_3[:, k, :], x_d[k * 128:(k + 1) * 128,
                                                     0:CA])
            # wih digit planes, per m-chunk (progressive: mm1 m-chunk can
            # start as soon as its planes land)
            wih_sb = []
            for m in range(KH):
                w = wih_pool.tile([128, KI * ND * 128], e5, tag=f"wih_{m}")
                nc.sync.dma_start(
                    w[:].rearrange("p (k f) -> p k f", k=KI),
                    wih_d[m * INPUT_DIM:(m + 1) * INPUT_DIM, :]
                    .rearrange("(k p) f -> p k f", p=128))
                wih_sb.append(w)

            y1 = st1_pool.tile([128, KH * BLOC], f32, tag="y1")
            nc.vector.memset(y1[:], 0.0)
            y1_3 = y1[:].rearrange("p (m b) -> p m b", m=KH)

            for blk in range(NBLK):
                c0 = blk * CA
                if blk == 0:
                    xt = x_first
                else:
                    xt = x_pool.tile([128, KI * CA], e5, tag="x")
                    xt_3 = xt[:].rearrange("p (k c) -> p k c", k=KI)
                    for k in range(KI):
                        nc.sync.dma_start(xt_3[:, k, :],
                                          x_d[k * 128:(k + 1) * 128,
                                              c0:c0 + CA])
                xt_3 = xt[:].rearrange("p (k c) -> p k c", k=KI)
                cur1 = cur1_pool.tile([128, KH * CA], f32, tag="cur1")
                for m in range(KH):
                    ps = psA.tile([128, CA], f32, tag="psA")
                    w4 = wih_sb[m][:].rearrange("p (k i m) -> p k i m",
                                                k=KI, i=ND)
                    for k in range(KI):
                        xb = xt_3[:, k, :].unsqueeze(1) \
                            .broadcast_to([128, 2, CA])
                        for j in range(ND // 2):
                            nc.tensor.matmul(
                                ps[:],
                                w4[:, k, 2 * j:2 * j + 2, :],
                                xb,
                                start=(k == 0 and j == 0),
                                stop=(k == KI - 1 and j == ND // 2 - 1),
                                perf_mode=DR)
                    nc.scalar.copy(cur1[:, m * CA:(m + 1) * CA], ps[:])
                cur1_r = cur1[:].rearrange("p (m c) -> p m c", m=KH)
                for t in range(TB):
                    # v computed in-place over the cur1 slice
                    v = cur1_r[:, :, t * BLOC:(t + 1) * BLOC]
                    ub = u1_3[:, :, c0 + t * BLOC:c0 + (t + 1) * BLOC]
                    # v = 0.9*y + cur
                    nc.vector.scalar_tensor_tensor(v, y1_3, DECAY, v,
                                                   op.mult, op.add)
                    # u = (v < 1) * 2^-14, e5m2 for the DoubleRow matmul
                    nc.vector.tensor_scalar(ub, v, TH, MOV,
                                            op.is_lt, op.mult)
                    # y = (v<1)*v
                    nc.vector.scalar_tensor_tensor(y1_3, v, TH, v,
                                                   op.is_lt, op.mult)

        # ---------------- Phase B: mm2 + LIF2 + mm3 + output scan -----------
        with tc.tile_pool(name="wst", bufs=2) as wst_pool, \
             tc.tile_pool(name="cur2", bufs=2) as cur2_pool, \
             tc.tile_pool(name="u2p", bufs=2) as u2_pool, \
             tc.tile_pool(name="smallB", bufs=1) as sm_pool, \
             tc.tile_pool(name="cur3", bufs=2) as cur3_pool, \
             tc.tile_pool(name="psB", bufs=6, space="PSUM") as psB, \
             tc.tile_pool(name="ps3", bufs=2, space="PSUM") as ps3_pool:

            who_sb = sm_pool.tile([128, KH * ND * 16], e5, tag="who")
            nc.sync.dma_start(
                who_sb[:].rearrange("p (k f) -> p k f", k=KH),
                who_d[:, :].rearrange("(k p) f -> p k f", p=128))
            who4 = who_sb[:].rearrange("p (k i m) -> p k i m", k=KH, i=ND)
            cs_hh = sm_pool.tile([128, KH], f32, tag="cshh")
            nc.sync.dma_start(cs_hh[:], cs_hh_d[:, :])
            cs_ho = sm_pool.tile([OUTPUT_DIM, 1], f32, tag="csho")
            nc.sync.dma_start(cs_ho[:], cs_ho_d[:, :])

            y2 = sm_pool.tile([128, KH * BLOC], f32, tag="y2")
            v2 = sm_pool.tile([128, KH * BLOC], f32, tag="v2")
            yo = sm_pool.tile([OUTPUT_DIM, BLOC], f32, tag="yo")
            vo = sm_pool.tile([OUTPUT_DIM, BLOC], f32, tag="vo")
            acc0 = sm_pool.tile([OUTPUT_DIM, BLOC], f32, tag="acc0")
            acc1 = sm_pool.tile([OUTPUT_DIM, BLOC], f32, tag="acc1")
            acc = [acc0, acc1]
            out_sb = sm_pool.tile([OUTPUT_DIM, BLOC], f32, tag="rate")
            nc.vector.memset(y2[:], 0.0)
            nc.vector.memset(yo[:], 0.0)
            nc.vector.memset(acc[0][:], 0.0)
            y2_3 = y2[:].rearrange("p (m b) -> p m b", m=KH)
            v2_3 = v2[:].rearrange("p (m b) -> p m b", m=KH)

            def emit_mm3(c0, sc, u2_3):
                """mm3 + output-layer scan for a finished superblock."""
                cur3 = cur3_pool.tile([OUTPUT_DIM, SCMAX], f32, tag="cur3")
                for nh in range(sc // C):
                    ps3 = ps3_pool.tile([OUTPUT_DIM, C], f32, tag="ps3")
                    for k in range(KH):
                        ub = u2_3[:, k, nh * C:(nh + 1) * C] \
                            .unsqueeze(1).broadcast_to([128, 2, C])
                        for j in range(ND // 2):
                            nc.tensor.matmul(
                                ps3[:],
                                who4[:, k, 2 * j:2 * j + 2, 0:OUTPUT_DIM],
                                ub,
                                start=(k == 0 and j == 0),
                                stop=(k == KH - 1 and j == ND // 2 - 1),
                                perf_mode=DR)
                    # cur3 = colsum_ho - u2@W_ho  (true output current)
                    nc.scalar.activation(cur3[:, nh * C:(nh + 1) * C],
                                         ps3[:], ident,
                                         bias=cs_ho[:, 0:1], scale=-1.0)
                for t in range(sc // BLOC):
                    g = c0 // BLOC + t
                    sl = cur3[:, t * BLOC:(t + 1) * BLOC]
                    nc.vector.scalar_tensor_tensor(vo[:], yo[:], DECAY, sl,
                                                   op.mult, op.add)
                    nc.vector.scalar_tensor_tensor(acc[(g + 1) % 2][:], vo[:],
                                                   TH, acc[g % 2][:],
                                                   op.is_lt, op.add)
                    nc.vector.scalar_tensor_tensor(yo[:], vo[:], TH, vo[:],
                                                   op.is_lt, op.mult)

            prev = None
            for c0, sc in SUPERS:
                cur2 = cur2_pool.tile([128, KH * SCMAX], f32, tag="cur2")
                u2 = u2_pool.tile([128, KH * SCMAX], e5, tag="u2")
                u2_3 = u2[:].rearrange("p (m c) -> p m c", m=KH)
                for m2 in range(KH):
                    if m2 == 8 and prev is not None:
                        emit_mm3(*prev)
                        prev = None
                    wst = wst_pool.tile([128, KH * ND * 128], e5, tag="wst")
                    nc.sync.dma_start(
                        wst[:].rearrange("p (k f) -> p k f", k=KH),
                        whh_d[m2 * HIDDEN_DIM:(m2 + 1) * HIDDEN_DIM, :]
                        .rearrange("(k p) f -> p k f", p=128))
                    wst4 = wst[:].rearrange("p (k i m) -> p k i m",
                                            k=KH, i=ND)
                    for nh in range(sc // C):
                        ps = psB.tile([128, C], f32, tag="psB")
                        for k in range(KH):
                            ub = u1_3[:, k,
                                      c0 + nh * C:c0 + (nh + 1) * C] \
                                .unsqueeze(1).broadcast_to([128, 2, C])
                            for j in range(ND // 2):
                                nc.tensor.matmul(
                                    ps[:],
                                    wst4[:, k, 2 * j:2 * j + 2, :],
                                    ub,
                                    start=(k == 0 and j == 0),
                                    stop=(k == KH - 1 and j == ND // 2 - 1),
                                    perf_mode=DR)
                        # cur2 = colsum_hh - u1@W_hh (true layer-2 current)
                        nc.scalar.activation(
                            cur2[:,
                                 m2 * SCMAX + nh * C:m2 * SCMAX + (nh + 1) * C],
                            ps[:], ident, bias=cs_hh[:, m2:m2 + 1],
                            scale=-1.0)
                cur2_r = cur2[:].rearrange("p (m c) -> p m c", m=KH)
                for t in range(sc // BLOC):
                    sl = cur2_r[:, :, t * BLOC:(t + 1) * BLOC]
                    ub = u2_3[:, :, t * BLOC:(t + 1) * BLOC]
                    nc.vector.scalar_tensor_tensor(v2_3, y2_3, DECAY, sl,
                                                   op.mult, op.add)
                    nc.vector.tensor_scalar(ub, v2_3, TH, MOV,
                                            op.is_lt, op.mult)
                    nc.vector.scalar_tensor_tensor(y2_3, v2_3, TH, v2_3,
                                                   op.is_lt, op.mult)
                prev = (c0, sc, u2_3)
            emit_mm3(*prev)

            # rate = 1 - acc/T   (acc holds sum of u_out; s = 1-u)
            nc.vector.tensor_scalar(out_sb[:], acc[T % 2][:], -1.0 / T, 1.0,
                                    op.mult, op.add)
            nc.sync.dma_start(out_d[:, :], out_sb[:])

    nc.compile()
    return nc


def _digit_planes(w):
    """Decompose fp32 weights into ND exact e5m2 digit planes.

    w ~= Wfix * 2^-KBITS with Wfix = sum_i d_i 16^i, d_i in [-8,7].
    Plane i holds d_i * 2^(4i - KBITS + 14); the moving operand carries
    2^-14, so plane_i * moving accumulates to exactly Wfix * 2^-KBITS.
    Returns (planes [ND, *w.shape] e5m2-exact fp32, effective weights fp32).
    """
    wfix = np.round(w.astype(np.float64) * (1 << KBITS)).astype(np.int64)
    assert np.abs(wfix).max() <= DMAX, "weights exceed digit range"
    planes = np.zeros((ND,) + w.shape, np.float32)
    rem = wfix.copy()
    for i in range(ND):
        d = ((rem + 8) % 16) - 8
        rem = (rem - d) >> 4
        planes[i] = d * np.float32(2.0 ** (4 * i - KBITS + 14))
    assert np.all(rem == 0)
    weff = (wfix * (2.0 ** -KBITS)).astype(np.float32)
    return planes, weff


def kernel(input_bins, W_ih, W_hh, W_ho):
    global _BUILT
    if _BUILT is None:
        _BUILT = _build()
    nc = _BUILT
    import ml_dtypes
    e5np = ml_dtypes.float8_e5m2

    input_bins = np.ascontiguousarray(input_bins, dtype=np.float32)
    W_ih = np.ascontiguousarray(W_ih, dtype=np.float32)
    W_hh2 = np.ascontiguousarray(np.asarray(W_hh)[0], dtype=np.float32)
    W_ho = np.ascontiguousarray(W_ho, dtype=np.float32)

    pih, wih_eff = _digit_planes(W_ih)       # [ND, 1024, 2048]
    phh, whh_eff = _digit_planes(W_hh2)      # [ND, 2048, 2048]
    pho, who_eff = _digit_planes(W_ho)       # [ND, 2048, 10]

    # wih planes -> [(m*KI + kt)*128, dig*128]
    wihd = np.ascontiguousarray(
        pih.reshape(ND, KI, 128, KH, 128)      # [dig, kt, p, m, mc]
        .transpose(3, 1, 2, 0, 4)              # [m, kt, p, dig, mc]
        .reshape(KH * INPUT_DIM, ND * 128)
    ).astype(e5np)
    # whh planes -> [(m2*KH + kt)*128, dig*128]
    whhd = np.ascontiguousarray(
        phh.reshape(ND, KH, 128, KH, 128)      # [dig, kt, p, m2, mc]
        .transpose(3, 1, 2, 0, 4)              # [m2, kt, p, dig, mc]
        .reshape(KH * HIDDEN_DIM, ND * 128)
    ).astype(e5np)
    # who planes padded to 16 output cols: [kt*128, dig*16]
    whod = np.zeros((KH, 128, ND, 16), np.float32)
    whod[:, :, :, :OUTPUT_DIM] = pho.reshape(ND, KH, 128, OUTPUT_DIM) \
        .transpose(1, 2, 0, 3)
    whod8 = np.ascontiguousarray(whod.reshape(KH * 128, ND * 16)).astype(e5np)

    cs_hh = np.ascontiguousarray(
        whh_eff.sum(axis=0, dtype=np.float64).astype(np.float32)
        .reshape(KH, 128).T)
    cs_ho = who_eff.sum(axis=0, dtype=np.float64).astype(np.float32) \
        .reshape(OUTPUT_DIM, 1)

    in_maps = []
    for c in range(NCORES):
        xb = input_bins[c * BLOC:(c + 1) * BLOC]        # [32, 1024, 100]
        xc = np.ascontiguousarray(
            xb.transpose(1, 2, 0).reshape(INPUT_DIM, COLS) * np.float32(MOV)
        ).astype(e5np)
        in_maps.append({
            "x": xc, "wihd": wihd, "whhd": whhd, "whod": whod8,
            "cs_hh": cs_hh, "cs_ho": cs_ho,
        })

    from concourse.bass_utils import run_bass_kernel_spmd
    res = run_bass_kernel_spmd(nc, in_maps, core_ids=list(range(NCORES)))

    out = np.empty((BATCH, OUTPUT_DIM), dtype=np.float32)
    for c in range(NCORES):
        out[c * BLOC:(c + 1) * BLOC] = res.results[c]["out"].T
    return out


# revision 18
# speedup vs baseline: 1.0447x; 1.0447x over previous
"""Trainium2 Bass kernel for a 2-hidden-layer LIF spiking network.

Math (per timestep t, per layer):
    v = 0.9*y + cur ;  spike s = (v >= 1) ;  y = v*(1-s) = v*u  with u = (v < 1)
Layer currents:
    cur1 = x_t @ W_ih            (x binary, precomputable for ALL t)
    cur2 = s1 @ W_hh = colsum(W_hh) - u1 @ W_hh
    cur3 = s2 @ W_ho = colsum(W_ho) - u2 @ W_ho
Output: rate = mean_t s_out = 1 - sum_t(u_out)/T

Key restructurings:
  * Layer 1's recurrence does not depend on layer 2, so all three matmuls are
    batched over the full (T*B) column space; only the cheap elementwise LIF
    scans are sequential in t.
  * Weights are quantized to 23-bit fixed point (step 2^-23) and decomposed
    into ND=6 exact signed base-16 digit planes, each stored in fp8 e5m2
    (digits in [-8,7] and power-of-2 scales are exact in e5m2). The moving
    operands (x and the spike complements u) carry the value 2^-14, exactly
    representable as the e5m2 minimum normal. Pairs of digit planes feed
    fp8 DoubleRow matmuls (2 stationary planes per instruction at 0.5
    cycles/row), so full 23-bit weight precision streams at 1.5 cycles/row
    vs 2.0 for an fp16 hi/lo pair -- with every product exact in fp32 PSUM.
  * The moving AP broadcasts the same spike tile across the DoubleRow pair
    (middle dim stride 0), so spikes are stored once, in SBUF, uncompressed
    (1 byte): the layer-1 spike complement for ALL timesteps stays resident
    and is never spilled to DRAM.

Sharding: data-parallel over batch (256/8 = 32 rows per core), weights
replicated, no cross-core communication.

Per-core schedule:
  Phase A (W_ih digit planes resident, 12.6MB, loaded progressively per
    output chunk so block 0 starts ~2.5us in): mm1 over blocks of 10 steps,
    LIF1 scan fused per block (v computed in-place over cur1), u1 written
    straight into its resident SBUF tile.
  Phase B (W_hh digit planes streamed from DRAM per 128-col output chunk,
    double-buffered): superblocks of 20 steps (last one split 2x10 so the
    final scan hides under mm2); mm2 -> cur2 with colsum correction fused
    into the PSUM->SBUF Identity-activation copy (scale=-1, bias=colsum),
    LIF2 scan, mm3 (emitted mid-way through the next superblock's mm2 so
    the PE never waits on the DVE scan), output LIF scan, final rate.
"""

import numpy as np

# ---- problem constants (hardcoded; kernel.py must be self-contained) ----
BATCH = 256
INPUT_DIM = 1024
HIDDEN_DIM = 2048
OUTPUT_DIM = 10
T = 100
NCORES = 8
BLOC = BATCH // NCORES          # 32 batch rows per core
TB = 5                          # timesteps per phase-A block
NBLK = T // TB                  # 20 blocks
CA = TB * BLOC                  # 160 columns per phase-A block
C = 320                         # columns per phase-B matmul chunk
COLS = T * BLOC                 # 3200 total columns
# phase-B superblocks (col0); u2 for super s lives in spike-buffer slot
# USLOT[s] (slot s-1 is dead once mm2(s) is done; slot 5 is spare for s=0)
SCMAX = 640
SUPERS = [0, 640, 1280, 1920, 2560]
USLOT = [5, 0, 1, 2, 3]
NSLOT = 6
KI = INPUT_DIM // 128           # 8 k-chunks for mm1
KH = HIDDEN_DIM // 128          # 16 k-chunks (and m-chunks) for mm2
DECAY = 0.9
THRESH = 1.0
TH_NUDGE = 0.0                  # tie-break re-roll knob (harmless ~1e-6 scale)

ND = 6                          # digit planes (23-bit fixed point)
KBITS = 23                      # weight step 2^-KBITS (max digit range 7829367)
MOV = 2.0 ** -14                # moving-operand value (e5m2 min normal)
DMAX = 7 * (16 ** ND - 1) // 15

_BUILT = None


def _build():
    """Trace + compile the Bass program once."""
    from contextlib import ExitStack

    import concourse.bacc as bacc
    import concourse.tile as tile
    from concourse import mybir
    from concourse.alu_op_type import AluOpType as op

    f32 = mybir.dt.float32
    e5 = mybir.dt.float8e5
    DR = mybir.MatmulPerfMode.DoubleRow
    ident = mybir.ActivationFunctionType.Identity
    TH = THRESH + TH_NUDGE

    nc = bacc.Bacc("TRN2", target_bir_lowering=False, debug=False,
                   num_devices=NCORES)

    # x values {0, 2^-14}: [input_dim, t*b] t-major columns
    x_d = nc.dram_tensor("x", [INPUT_DIM, COLS], e5,
                         kind="ExternalInput").ap()
    # wih digit planes, m-chunk major: [(m*KI + kt)*128, dig*128]
    wih_d = nc.dram_tensor("wihd", [KH * INPUT_DIM, ND * 128], e5,
                           kind="ExternalInput").ap()
    # whh digit planes: [(m2*KH + kt)*128, dig*128]
    whh_d = nc.dram_tensor("whhd", [KH * HIDDEN_DIM, ND * 128], e5,
                           kind="ExternalInput").ap()
    # who planes padded to 16 cols: [kt*128, dig*16]
    who_d = nc.dram_tensor("whod", [KH * 128, ND * 16], e5,
                           kind="ExternalInput").ap()
    cs_hh_d = nc.dram_tensor("cs_hh", [128, KH], f32, kind="ExternalInput").ap()
    cs_ho_d = nc.dram_tensor("cs_ho", [OUTPUT_DIM, 1], f32,
                             kind="ExternalInput").ap()
    out_d = nc.dram_tensor("out", [OUTPUT_DIM, BLOC], f32,
                           kind="ExternalOutput").ap()

    with tile.TileContext(nc) as tc, ExitStack() as ctx:
        # spike complements {0, 2^-14}, resident across both phases
        # [p, kt, col]: cols 0..3200 hold u1; 6 ring slots of 640 also serve
        # as u2 staging (a slot is reused once mm2 has consumed its u1 cols)
        u1_pool = ctx.enter_context(tc.tile_pool(name="u1", bufs=1))
        u1 = u1_pool.tile([128, KH * NSLOT * SCMAX], e5, tag="u1")
        u1_3 = u1[:].rearrange("p (k c) -> p k c", k=KH)

        # ---------------- Phase A: mm1 + LIF1 scan ----------------
        with tc.tile_pool(name="wih", bufs=1) as wih_pool, \
             tc.tile_pool(name="xin", bufs=2) as x_pool, \
             tc.tile_pool(name="cur1", bufs=2) as cur1_pool, \
             tc.tile_pool(name="st1", bufs=1) as st1_pool, \
             tc.tile_pool(name="psA", bufs=6, space="PSUM") as psA:

            # x for block 0 first so the DMA queue starts useful work early
            x_first = x_pool.tile([128, KI * CA], e5, tag="x")
            xf_3 = x_first[:].rearrange("p (k c) -> p k c", k=KI)
            for k in range(KI):
                nc.sync.dma_start(xf_3[:, k, :], x_d[k * 128:(k + 1) * 128,
                                                     0:CA])
            # wih digit planes, per m-chunk (progressive: mm1 m-chunk can
            # start as soon as its planes land)
            wih_sb = []
            for m in range(KH):
                w = wih_pool.tile([128, KI * ND * 128], e5, tag=f"wih_{m}")
                nc.sync.dma_start(
                    w[:].rearrange("p (k f) -> p k f", k=KI),
                    wih_d[m * INPUT_DIM:(m + 1) * INPUT_DIM, :]
                    .rearrange("(k p) f -> p k f", p=128))
                wih_sb.append(w)

            y1 = st1_pool.tile([128, KH * BLOC], f32, tag="y1")
            nc.vector.memset(y1[:], 0.0)
            y1_3 = y1[:].rearrange("p (m b) -> p m b", m=KH)

            for blk in range(NBLK):
                c0 = blk * CA
                if blk == 0:
                    xt = x_first
                else:
                    xt = x_pool.tile([128, KI * CA], e5, tag="x")
                    xt_3 = xt[:].rearrange("p (k c) -> p k c", k=KI)
                    for k in range(KI):
                        nc.sync.dma_start(xt_3[:, k, :],
                                          x_d[k * 128:(k + 1) * 128,
                                              c0:c0 + CA])
                xt_3 = xt[:].rearrange("p (k c) -> p k c", k=KI)
                cur1 = cur1_pool.tile([128, KH * CA], f32, tag="cur1")
                for m in range(KH):
                    ps = psA.tile([128, CA], f32, tag="psA")
                    w4 = wih_sb[m][:].rearrange("p (k i m) -> p k i m",
                                                k=KI, i=ND)
                    for k in range(KI):
                        xb = xt_3[:, k, :].unsqueeze(1) \
                            .broadcast_to([128, 2, CA])
                        for j in range(ND // 2):
                            nc.tensor.matmul(
                                ps[:],
                                w4[:, k, 2 * j:2 * j + 2, :],
                                xb,
                                start=(k == 0 and j == 0),
                                stop=(k == KI - 1 and j == ND // 2 - 1),
                                perf_mode=DR)
                    nc.scalar.copy(cur1[:, m * CA:(m + 1) * CA], ps[:])
                cur1_r = cur1[:].rearrange("p (m c) -> p m c", m=KH)
                for t in range(TB):
                    # v computed in-place over the cur1 slice
                    v = cur1_r[:, :, t * BLOC:(t + 1) * BLOC]
                    ub = u1_3[:, :, c0 + t * BLOC:c0 + (t + 1) * BLOC]
                    # v = 0.9*y + cur
                    nc.vector.scalar_tensor_tensor(v, y1_3, DECAY, v,
                                                   op.mult, op.add)
                    # u = (v < 1) * 2^-14, e5m2 for the DoubleRow matmul
                    nc.vector.tensor_scalar(ub, v, TH, MOV,
                                            op.is_lt, op.mult)
                    # y = (v<1)*v
                    nc.vector.scalar_tensor_tensor(y1_3, v, TH, v,
                                                   op.is_lt, op.mult)

        # ---------------- Phase B: mm2 + LIF2 + mm3 + output scan -----------
        with tc.tile_pool(name="wst", bufs=3) as wst_pool, \
             tc.tile_pool(name="cur2", bufs=2) as cur2_pool, \
             tc.tile_pool(name="smallB", bufs=1) as sm_pool, \
             tc.tile_pool(name="cur3", bufs=1) as cur3_pool, \
             tc.tile_pool(name="psB", bufs=6, space="PSUM") as psB, \
             tc.tile_pool(name="ps3", bufs=2, space="PSUM") as ps3_pool:

            who_sb = sm_pool.tile([128, KH * ND * 16], e5, tag="who")
            nc.sync.dma_start(
                who_sb[:].rearrange("p (k f) -> p k f", k=KH),
                who_d[:, :].rearrange("(k p) f -> p k f", p=128))
            who4 = who_sb[:].rearrange("p (k i m) -> p k i m", k=KH, i=ND)
            cs_hh = sm_pool.tile([128, KH], f32, tag="cshh")
            nc.sync.dma_start(cs_hh[:], cs_hh_d[:, :])
            cs_ho = sm_pool.tile([OUTPUT_DIM, 1], f32, tag="csho")
            nc.sync.dma_start(cs_ho[:], cs_ho_d[:, :])

            y2 = sm_pool.tile([128, KH * BLOC], f32, tag="y2")
            yo = sm_pool.tile([OUTPUT_DIM, BLOC], f32, tag="yo")
            vo = sm_pool.tile([OUTPUT_DIM, BLOC], f32, tag="vo")
            acc0 = sm_pool.tile([OUTPUT_DIM, BLOC], f32, tag="acc0")
            acc1 = sm_pool.tile([OUTPUT_DIM, BLOC], f32, tag="acc1")
            acc = [acc0, acc1]
            out_sb = sm_pool.tile([OUTPUT_DIM, BLOC], f32, tag="rate")
            nc.vector.memset(y2[:], 0.0)
            nc.vector.memset(yo[:], 0.0)
            nc.vector.memset(acc[0][:], 0.0)
            y2_3 = y2[:].rearrange("p (m b) -> p m b", m=KH)

            def emit_mm3(c0, uoff):
                """mm3 + output-layer scan for a finished superblock."""
                cur3 = cur3_pool.tile([OUTPUT_DIM, SCMAX], f32, tag="cur3")
                for nh in range(SCMAX // C):
                    ps3 = ps3_pool.tile([OUTPUT_DIM, C], f32, tag="ps3")
                    for k in range(KH):
                        ub = u1_3[:, k, uoff + nh * C:uoff + (nh + 1) * C] \
                            .unsqueeze(1).broadcast_to([128, 2, C])
                        for j in range(ND // 2):
                            nc.tensor.matmul(
                                ps3[:],
                                who4[:, k, 2 * j:2 * j + 2, 0:OUTPUT_DIM],
                                ub,
                                start=(k == 0 and j == 0),
                                stop=(k == KH - 1 and j == ND // 2 - 1),
                                perf_mode=DR)
                    # cur3 = colsum_ho - u2@W_ho  (true output current)
                    nc.scalar.activation(cur3[:, nh * C:(nh + 1) * C],
                                         ps3[:], ident,
                                         bias=cs_ho[:, 0:1], scale=-1.0)
                for t in range(SCMAX // BLOC):
                    g = c0 // BLOC + t
                    sl = cur3[:, t * BLOC:(t + 1) * BLOC]
                    nc.vector.scalar_tensor_tensor(vo[:], yo[:], DECAY, sl,
                                                   op.mult, op.add)
                    nc.vector.scalar_tensor_tensor(acc[(g + 1) % 2][:], vo[:],
                                                   TH, acc[g % 2][:],
                                                   op.is_lt, op.add)
                    nc.vector.scalar_tensor_tensor(yo[:], vo[:], TH, vo[:],
                                                   op.is_lt, op.mult)

            prev = None
            for sup, c0 in enumerate(SUPERS):
                uoff = USLOT[sup] * SCMAX
                cur2 = cur2_pool.tile([128, KH * SCMAX], f32, tag="cur2")
                for m2 in range(KH):
                    if m2 == 8 and prev is not None:
                        emit_mm3(*prev)
                        prev = None
                    wst = wst_pool.tile([128, KH * ND * 128], e5, tag="wst")
                    nc.sync.dma_start(
                        wst[:].rearrange("p (k f) -> p k f", k=KH),
                        whh_d[m2 * HIDDEN_DIM:(m2 + 1) * HIDDEN_DIM, :]
                        .rearrange("(k p) f -> p k f", p=128))
                    wst4 = wst[:].rearrange("p (k i m) -> p k i m",
                                            k=KH, i=ND)
                    for nh in range(SCMAX // C):
                        ps = psB.tile([128, C], f32, tag="psB")
                        for k in range(KH):
                            ub = u1_3[:, k,
                                      c0 + nh * C:c0 + (nh + 1) * C] \
                                .unsqueeze(1).broadcast_to([128, 2, C])
                            for j in range(ND // 2):
                                nc.tensor.matmul(
                                    ps[:],
                                    wst4[:, k, 2 * j:2 * j + 2, :],
                                    ub,
                                    start=(k == 0 and j == 0),
                                    stop=(k == KH - 1 and j == ND // 2 - 1),
                                    perf_mode=DR)
                        # cur2 = colsum_hh - u1@W_hh (true layer-2 current)
                        nc.scalar.activation(
                            cur2[:,
                                 m2 * SCMAX + nh * C:m2 * SCMAX + (nh + 1) * C],
                            ps[:], ident, bias=cs_hh[:, m2:m2 + 1],
                            scale=-1.0)
                cur2_r = cur2[:].rearrange("p (m c) -> p m c", m=KH)
                for t in range(SCMAX // BLOC):
                    # v computed in-place over the cur2 slice
                    v = cur2_r[:, :, t * BLOC:(t + 1) * BLOC]
                    ub = u1_3[:, :, uoff + t * BLOC:uoff + (t + 1) * BLOC]
                    nc.vector.scalar_tensor_tensor(v, y2_3, DECAY, v,
                                                   op.mult, op.add)
                    nc.vector.tensor_scalar(ub, v, TH, MOV,
                                            op.is_lt, op.mult)
                    nc.vector.scalar_tensor_tensor(y2_3, v, TH, v,
                                                   op.is_lt, op.mult)
                prev = (c0, uoff)
            emit_mm3(*prev)

            # rate = 1 - acc/T   (acc holds sum of u_out; s = 1-u)
            nc.vector.tensor_scalar(out_sb[:], acc[T % 2][:], -1.0 / T, 1.0,
                                    op.mult, op.add)
            nc.sync.dma_start(out_d[:, :], out_sb[:])

    nc.compile()
    return nc


def _digit_planes(w):
    """Decompose fp32 weights into ND exact e5m2 digit planes.

    w ~= Wfix * 2^-KBITS with Wfix = sum_i d_i 16^i, d_i in [-8,7].
    Plane i holds d_i * 2^(4i - KBITS + 14); the moving operand carries
    2^-14, so plane_i * moving accumulates to exactly Wfix * 2^-KBITS.
    Returns (planes [ND, *w.shape] e5m2-exact fp32, effective weights fp32).
    """
    wfix = np.round(w.astype(np.float64) * (1 << KBITS)).astype(np.int64)
    assert np.abs(wfix).max() <= DMAX, "weights exceed digit range"
    planes = np.zeros((ND,) + w.shape, np.float32)
    rem = wfix.copy()
    for i in range(ND):
        d = ((rem + 8) % 16) - 8
        rem = (rem - d) >> 4
        planes[i] = d * np.float32(2.0 ** (4 * i - KBITS + 14))
    assert np.all(rem == 0)
    weff = (wfix * (2.0 ** -KBITS)).astype(np.float32)
    return planes, weff


def kernel(input_bins, W_ih, W_hh, W_ho):
    global _BUILT
    if _BUILT is None:
        _BUILT = _build()
    nc = _BUILT
    import ml_dtypes
    e5np = ml_dtypes.float8_e5m2

    input_bins = np.ascontiguousarray(input_bins, dtype=np.float32)
    W_ih = np.ascontiguousarray(W_ih, dtype=np.float32)
    W_hh2 = np.ascontiguousarray(np.asarray(W_hh)[0], dtype=np.float32)
    W_ho = np.ascontiguousarray(W_ho, dtype=np.float32)

    pih, wih_eff = _digit_planes(W_ih)       # [ND, 1024, 2048]
    phh, whh_eff = _digit_planes(W_hh2)      # [ND, 2048, 2048]
    pho, who_eff = _digit_planes(W_ho)       # [ND, 2048, 10]

    # wih planes -> [(m*KI + kt)*128, dig*128]
    wihd = np.ascontiguousarray(
        pih.reshape(ND, KI, 128, KH, 128)      # [dig, kt, p, m, mc]
        .transpose(3, 1, 2, 0, 4)              # [m, kt, p, dig, mc]
        .reshape(KH * INPUT_DIM, ND * 128)
    ).astype(e5np)
    # whh planes -> [(m2*KH + kt)*128, dig*128]
    whhd = np.ascontiguousarray(
        phh.reshape(ND, KH, 128, KH, 128)      # [dig, kt, p, m2, mc]
        .transpose(3, 1, 2, 0, 4)              # [m2, kt, p, dig, mc]
        .reshape(KH * HIDDEN_DIM, ND * 128)
    ).astype(e5np)
    # who planes padded to 16 output cols: [kt*128, dig*16]
    whod = np.zeros((KH, 128, ND, 16), np.float32)
    whod[:, :, :, :OUTPUT_DIM] = pho.reshape(ND, KH, 128, OUTPUT_DIM) \
        .transpose(1, 2, 0, 3)
    whod8 = np.ascontiguousarray(whod.reshape(KH * 128, ND * 16)).astype(e5np)

    cs_hh = np.ascontiguousarray(
        whh_eff.sum(axis=0, dtype=np.float64).astype(np.float32)
        .reshape(KH, 128).T)
    cs_ho = who_eff.sum(axis=0, dtype=np.float64).astype(np.float32) \
        .reshape(OUTPUT_DIM, 1)

    in_maps = []
    for c in range(NCORES):
        xb = input_bins[c * BLOC:(c + 1) * BLOC]        # [32, 1024, 100]
        xc = np.ascontiguousarray(
            xb.transpose(1, 2, 0).reshape(INPUT_DIM, COLS) * np.float32(MOV)
        ).astype(e5np)
        in_maps.append({
            "x": xc, "wihd": wihd, "whhd": whhd, "whod": whod8,
            "cs_hh": cs_hh, "cs_ho": cs_ho,
        })

    from concourse.bass_utils import run_bass_kernel_spmd
    res = run_bass_kernel_spmd(nc, in_maps, core_ids=list(range(NCORES)))

    out = np.empty((BATCH, OUTPUT_DIM), dtype=np.float32)
    for c in range(NCORES):
        out[c * BLOC:(c + 1) * BLOC] = res.results[c]["out"].T
    return out


# revision 28
# speedup vs baseline: 1.0601x; 1.0147x over previous
"""Trainium2 Bass kernel for a 2-hidden-layer LIF spiking network.

Math (per timestep t, per layer):
    v = 0.9*y + cur ;  spike s = (v >= 1) ;  y = v*(1-s) = v*u  with u = (v < 1)
Layer currents:
    cur1 = x_t @ W_ih            (x binary, precomputable for ALL t)
    cur2 = s1 @ W_hh = colsum(W_hh) - u1 @ W_hh
    cur3 = s2 @ W_ho = colsum(W_ho) - u2 @ W_ho
Output: rate = mean_t s_out = 1 - sum_t(u_out)/T

Key restructurings:
  * Layer 1's recurrence does not depend on layer 2, so all three matmuls are
    batched over the full (T*B) column space; only the cheap elementwise LIF
    scans are sequential in t.
  * Weights are quantized to 23-bit fixed point (step 2^-23) and decomposed
    into ND=6 exact signed base-16 digit planes, each stored in fp8 e5m2
    (digits in [-8,7] and power-of-2 scales are exact in e5m2). The moving
    operands (x and the spike complements u) carry the value 2^-14, exactly
    representable as the e5m2 minimum normal. Pairs of digit planes feed
    fp8 DoubleRow matmuls (2 stationary planes per instruction at 0.5
    cycles/row), so full 23-bit weight precision streams at 1.5 cycles/row
    vs 2.0 for an fp16 hi/lo pair -- with every product exact in fp32 PSUM.
  * The moving AP broadcasts the same spike tile across the DoubleRow pair
    (middle dim stride 0), so spikes are stored once, in SBUF, uncompressed
    (1 byte): the layer-1 spike complement for ALL timesteps stays resident
    and is never spilled to DRAM.

Sharding: data-parallel over batch (256/8 = 32 rows per core), weights
replicated, no cross-core communication.

Per-core schedule:
  Phase A (W_ih digit planes resident, 12.6MB, loaded progressively per
    output chunk so block 0 starts ~2.5us in): mm1 over blocks of 10 steps,
    LIF1 scan fused per block (v computed in-place over cur1), u1 written
    straight into its resident SBUF tile.
  Phase B (W_hh digit planes streamed from DRAM per 128-col output chunk,
    double-buffered): superblocks of 20 steps (last one split 2x10 so the
    final scan hides under mm2); mm2 -> cur2 with colsum correction fused
    into the PSUM->SBUF Identity-activation copy (scale=-1, bias=colsum),
    LIF2 scan, mm3 (emitted mid-way through the next superblock's mm2 so
    the PE never waits on the DVE scan), output LIF scan, final rate.
"""

import numpy as np

# ---- problem constants (hardcoded; kernel.py must be self-contained) ----
BATCH = 256
INPUT_DIM = 1024
HIDDEN_DIM = 2048
OUTPUT_DIM = 10
T = 100
NCORES = 8
BLOC = BATCH // NCORES          # 32 batch rows per core
TB = 5                          # timesteps per phase-A block
NBLK = T // TB                  # 20 blocks
CA = TB * BLOC                  # 160 columns per phase-A block
C = 320                         # columns per phase-B matmul chunk
COLS = T * BLOC                 # 3200 total columns
# phase-B superblocks (col0); u2 for super s lives in spike-buffer slot
# USLOT[s] (slot s-1 is dead once mm2(s) is done; slot 5 is spare for s=0)
SCMAX = 640
SUPERS = [0, 640, 1280, 1920, 2560]
USLOT = [5, 0, 1, 2, 3]
NSLOT = 6
KI = INPUT_DIM // 128           # 8 k-chunks for mm1
KH = HIDDEN_DIM // 128          # 16 k-chunks (and m-chunks) for mm2
DECAY = 0.9
THRESH = 1.0
TH_NUDGE = 0.0                  # tie-break re-roll knob (harmless ~1e-6 scale)

ND = 6                          # digit planes (23-bit fixed point)
KBITS = 23                      # weight step 2^-KBITS (max digit range 7829367)
MOV = 2.0 ** -14                # moving-operand value (e5m2 min normal)
DMAX = 7 * (16 ** ND - 1) // 15

_BUILT = None


def _build():
    """Trace + compile the Bass program once."""
    from contextlib import ExitStack

    import concourse.bacc as bacc
    import concourse.tile as tile
    from concourse import mybir
    from concourse.alu_op_type import AluOpType as op

    f32 = mybir.dt.float32
    e5 = mybir.dt.float8e5
    DR = mybir.MatmulPerfMode.DoubleRow
    ident = mybir.ActivationFunctionType.Identity
    TH = THRESH + TH_NUDGE

    nc = bacc.Bacc("TRN2", target_bir_lowering=False, debug=False,
                   num_devices=NCORES)

    # x values {0, 2^-14}: [input_dim, t*b] t-major columns
    x_d = nc.dram_tensor("x", [INPUT_DIM, COLS], e5,
                         kind="ExternalInput").ap()
    # wih digit planes, m-chunk major: [(m*KI + kt)*128, dig*128]
    wih_d = nc.dram_tensor("wihd", [KH * INPUT_DIM, ND * 128], e5,
                           kind="ExternalInput").ap()
    # whh digit planes: [(m2*KH + kt)*128, dig*128]
    whh_d = nc.dram_tensor("whhd", [KH * HIDDEN_DIM, ND * 128], e5,
                           kind="ExternalInput").ap()
    # who planes padded to 16 cols: [kt*128, dig*16]
    who_d = nc.dram_tensor("whod", [KH * 128, ND * 16], e5,
                           kind="ExternalInput").ap()
    cs_hh_d = nc.dram_tensor("cs_hh", [128, KH], f32, kind="ExternalInput").ap()
    cs_ho_d = nc.dram_tensor("cs_ho", [OUTPUT_DIM, 1], f32,
                             kind="ExternalInput").ap()
    id_d = nc.dram_tensor("ident", [128, 128], f32, kind="ExternalInput").ap()
    out_d = nc.dram_tensor("out", [OUTPUT_DIM, BLOC], f32,
                           kind="ExternalOutput").ap()

    with tile.TileContext(nc) as tc, ExitStack() as ctx:
        # spike complements {0, 2^-14}, resident across both phases
        # [p, kt, col]: cols 0..3200 hold u1; 6 ring slots of 640 also serve
        # as u2 staging (a slot is reused once mm2 has consumed its u1 cols)
        u1_pool = ctx.enter_context(tc.tile_pool(name="u1", bufs=1))
        u1 = u1_pool.tile([128, KH * NSLOT * SCMAX], e5, tag="u1")
        u1_3 = u1[:].rearrange("p (k c) -> p k c", k=KH)

        # ---------------- Phase A: mm1 + LIF1 scan ----------------
        with tc.tile_pool(name="wih", bufs=1) as wih_pool, \
             tc.tile_pool(name="xin", bufs=2) as x_pool, \
             tc.tile_pool(name="cur1", bufs=2) as cur1_pool, \
             tc.tile_pool(name="st1", bufs=1) as st1_pool, \
             tc.tile_pool(name="psA", bufs=6, space="PSUM") as psA:

            # x for block 0 first so the DMA queue starts useful work early
            x_first = x_pool.tile([128, KI * CA], e5, tag="x")
            xf_3 = x_first[:].rearrange("p (k c) -> p k c", k=KI)
            for k in range(KI):
                nc.sync.dma_start(xf_3[:, k, :], x_d[k * 128:(k + 1) * 128,
                                                     0:CA])
            # wih digit planes, per m-chunk (progressive: mm1 m-chunk can
            # start as soon as its planes land)
            wih_sb = []
            for m in range(KH):
                w = wih_pool.tile([128, KI * ND * 128], e5, tag=f"wih_{m}")
                nc.sync.dma_start(
                    w[:].rearrange("p (k f) -> p k f", k=KI),
                    wih_d[m * INPUT_DIM:(m + 1) * INPUT_DIM, :]
                    .rearrange("(k p) f -> p k f", p=128))
                wih_sb.append(w)

            y1 = st1_pool.tile([128, KH * BLOC], f32, tag="y1")
            nc.vector.memset(y1[:], 0.0)
            y1_3 = y1[:].rearrange("p (m b) -> p m b", m=KH)

            def mm1_block(xt_3, cur1, m):
                ps = psA.tile([128, CA], f32, tag="psA")
                w4 = wih_sb[m][:].rearrange("p (k i m) -> p k i m",
                                            k=KI, i=ND)
                for k in range(KI):
                    xb = xt_3[:, k, :].unsqueeze(1) \
                        .broadcast_to([128, 2, CA])
                    for j in range(ND // 2):
                        nc.tensor.matmul(
                            ps[:],
                            w4[:, k, 2 * j:2 * j + 2, :],
                            xb,
                            start=(k == 0 and j == 0),
                            stop=(k == KI - 1 and j == ND // 2 - 1),
                            perf_mode=DR)
                nc.scalar.copy(cur1[:, m * CA:(m + 1) * CA], ps[:])

            def scan1_block(cur1, c0):
                cur1_r = cur1[:].rearrange("p (m c) -> p m c", m=KH)
                for t in range(TB):
                    # v computed in-place over the cur1 slice
                    v = cur1_r[:, :, t * BLOC:(t + 1) * BLOC]
                    ub = u1_3[:, :, c0 + t * BLOC:c0 + (t + 1) * BLOC]
                    # v = 0.9*y + cur
                    nc.vector.scalar_tensor_tensor(v, y1_3, DECAY, v,
                                                   op.mult, op.add)
                    # u = (v < 1) * 2^-14, e5m2 for the DoubleRow matmul
                    nc.vector.tensor_scalar(ub, v, TH, MOV,
                                            op.is_lt, op.mult)
                    # y = (v<1)*v
                    nc.vector.scalar_tensor_tensor(y1_3, v, TH, v,
                                                   op.is_lt, op.mult)

            # blocks 0+1 interleaved per m-chunk: mm1 rides the progressive
            # wih DMA (one m-chunk's planes feed both blocks back to back)
            x_b1 = x_pool.tile([128, KI * CA], e5, tag="x")
            xb1_3 = x_b1[:].rearrange("p (k c) -> p k c", k=KI)
            for k in range(KI):
                nc.sync.dma_start(xb1_3[:, k, :],
                                  x_d[k * 128:(k + 1) * 128, CA:2 * CA])
            cur1_b0 = cur1_pool.tile([128, KH * CA], f32, tag="cur1")
            cur1_b1 = cur1_pool.tile([128, KH * CA], f32, tag="cur1")
            for m in range(KH):
                mm1_block(xf_3, cur1_b0, m)
                mm1_block(xb1_3, cur1_b1, m)
            scan1_block(cur1_b0, 0)
            scan1_block(cur1_b1, CA)

            for blk in range(2, NBLK):
                c0 = blk * CA
                xt = x_pool.tile([128, KI * CA], e5, tag="x")
                xt_3 = xt[:].rearrange("p (k c) -> p k c", k=KI)
                for k in range(KI):
                    nc.sync.dma_start(xt_3[:, k, :],
                                      x_d[k * 128:(k + 1) * 128,
                                          c0:c0 + CA])
                cur1 = cur1_pool.tile([128, KH * CA], f32, tag="cur1")
                for m in range(KH):
                    mm1_block(xt_3, cur1, m)
                scan1_block(cur1, c0)

        # ---------------- Phase B: mm2 + LIF2 + mm3 + output scan -----------
        with tc.tile_pool(name="wst", bufs=3) as wst_pool, \
             tc.tile_pool(name="cur2", bufs=2) as cur2_pool, \
             tc.tile_pool(name="smallB", bufs=1) as sm_pool, \
             tc.tile_pool(name="cur3", bufs=1) as cur3_pool, \
             tc.tile_pool(name="s3p", bufs=2) as s3_pool, \
             tc.tile_pool(name="psB", bufs=4, space="PSUM") as psB, \
             tc.tile_pool(name="ps3", bufs=2, space="PSUM") as ps3_pool, \
             tc.tile_pool(name="pstr", bufs=2, space="PSUM") as pstr_pool:

            who_sb = sm_pool.tile([128, KH * ND * 16], e5, tag="who")
            nc.sync.dma_start(
                who_sb[:].rearrange("p (k f) -> p k f", k=KH),
                who_d[:, :].rearrange("(k p) f -> p k f", p=128))
            who4 = who_sb[:].rearrange("p (k i m) -> p k i m", k=KH, i=ND)
            cs_hh = sm_pool.tile([128, KH], f32, tag="cshh")
            nc.sync.dma_start(cs_hh[:], cs_hh_d[:, :])
            cs_ho = sm_pool.tile([OUTPUT_DIM, 1], f32, tag="csho")
            nc.sync.dma_start(cs_ho[:], cs_ho_d[:, :])
            ident_sb = sm_pool.tile([128, 128], f32, tag="ident")
            nc.sync.dma_start(ident_sb[:], id_d[:, :])

            y2 = sm_pool.tile([128, KH * BLOC], f32, tag="y2")
            yo = sm_pool.tile([OUTPUT_DIM, BLOC], f32, tag="yo")
            vo = sm_pool.tile([OUTPUT_DIM, BLOC], f32, tag="vo")
            acc0 = sm_pool.tile([OUTPUT_DIM, BLOC], f32, tag="acc0")
            acc1 = sm_pool.tile([OUTPUT_DIM, BLOC], f32, tag="acc1")
            acc = [acc0, acc1]
            out_sb = sm_pool.tile([OUTPUT_DIM, BLOC], f32, tag="rate")
            nc.vector.memset(y2[:], 0.0)
            nc.vector.memset(yo[:], 0.0)
            nc.vector.memset(acc[0][:], 0.0)
            y2_3 = y2[:].rearrange("p (m b) -> p m b", m=KH)

            def emit_mm3(c0, uoff):
                """mm3 (transposed: moving side = W_ho planes, 10-wide
                output) + PE transpose back + output-layer scan."""
                cur3 = cur3_pool.tile([OUTPUT_DIM, SCMAX], f32, tag="cur3")
                for ch in range(SCMAX // 128):
                    ps3 = ps3_pool.tile([128, OUTPUT_DIM], f32, tag="ps3")
                    for k in range(KH // 2):
                        # stationary: u2 k-tile pair; moving: W_ho planes
                        ub = u1_3[:, 2 * k:2 * k + 2,
                                  uoff + ch * 128:uoff + (ch + 1) * 128]
                        for i in range(ND):
                            nc.tensor.matmul(
                                ps3[:],
                                ub,
                                who4[:, 2 * k:2 * k + 2, i,
                                     0:OUTPUT_DIM],
                                start=(k == 0 and i == 0),
                                stop=(k == KH // 2 - 1 and i == ND - 1),
                                perf_mode=DR)
                    s3 = s3_pool.tile([128, OUTPUT_DIM], f32, tag="s3")
                    nc.scalar.copy(s3[:], ps3[:])
                    pst = pstr_pool.tile([OUTPUT_DIM, 128], f32, tag="pst")
                    nc.tensor.transpose(pst[:], s3[:], ident_sb[:])
                    # cur3 = colsum_ho - u2@W_ho  (true output current)
                    nc.scalar.activation(cur3[:, ch * 128:(ch + 1) * 128],
                                         pst[:], ident,
                                         bias=cs_ho[:, 0:1], scale=-1.0)
                for t in range(SCMAX // BLOC):
                    g = c0 // BLOC + t
                    sl = cur3[:, t * BLOC:(t + 1) * BLOC]
                    nc.vector.scalar_tensor_tensor(vo[:], yo[:], DECAY, sl,
                                                   op.mult, op.add)
                    nc.vector.scalar_tensor_tensor(acc[(g + 1) % 2][:], vo[:],
                                                   TH, acc[g % 2][:],
                                                   op.is_lt, op.add)
                    nc.vector.scalar_tensor_tensor(yo[:], vo[:], TH, vo[:],
                                                   op.is_lt, op.mult)

            prev = None
            for sup, c0 in enumerate(SUPERS):
                uoff = USLOT[sup] * SCMAX
                cur2 = cur2_pool.tile([128, KH * SCMAX], f32, tag="cur2")
                for m2 in range(KH):
                    if m2 == 8 and prev is not None:
                        emit_mm3(*prev)
                        prev = None
                    wst = wst_pool.tile([128, KH * ND * 128], e5, tag="wst")
                    nc.sync.dma_start(
                        wst[:].rearrange("p (k f) -> p k f", k=KH),
                        whh_d[m2 * HIDDEN_DIM:(m2 + 1) * HIDDEN_DIM, :]
                        .rearrange("(k p) f -> p k f", p=128))
                    wst4 = wst[:].rearrange("p (k i m) -> p k i m",
                                            k=KH, i=ND)
                    for nh in range(SCMAX // C):
                        ps = psB.tile([128, C], f32, tag="psB")
                        for k in range(KH):
                            ub = u1_3[:, k,
                                      c0 + nh * C:c0 + (nh + 1) * C] \
                                .unsqueeze(1).broadcast_to([128, 2, C])
                            for j in range(ND // 2):
                                nc.tensor.matmul(
                                    ps[:],
                                    wst4[:, k, 2 * j:2 * j + 2, :],
                                    ub,
                                    start=(k == 0 and j == 0),
                                    stop=(k == KH - 1 and j == ND // 2 - 1),
                                    perf_mode=DR)
                        # cur2 = colsum_hh - u1@W_hh (true layer-2 current)
                        nc.scalar.activation(
                            cur2[:,
                                 m2 * SCMAX + nh * C:m2 * SCMAX + (nh + 1) * C],
                            ps[:], ident, bias=cs_hh[:, m2:m2 + 1],
                            scale=-1.0)
                cur2_r = cur2[:].rearrange("p (m c) -> p m c", m=KH)
                parts = [(nc.vector, 0, KH)]
                for t in range(SCMAX // BLOC):
                    for eng, ml, mh in parts:
                        # v computed in-place over the cur2 slice
                        v = cur2_r[:, ml:mh, t * BLOC:(t + 1) * BLOC]
                        ub = u1_3[:, ml:mh,
                                  uoff + t * BLOC:uoff + (t + 1) * BLOC]
                        yy = y2_3[:, ml:mh, :]
                        eng.scalar_tensor_tensor(v, yy, DECAY, v,
                                                 op.mult, op.add)
                        eng.tensor_scalar(ub, v, TH, MOV,
                                          op.is_lt, op.mult)
                        eng.scalar_tensor_tensor(yy, v, TH, v,
                                                 op.is_lt, op.mult)
                prev = (c0, uoff)
            emit_mm3(*prev)

            # rate = 1 - acc/T   (acc holds sum of u_out; s = 1-u)
            nc.vector.tensor_scalar(out_sb[:], acc[T % 2][:], -1.0 / T, 1.0,
                                    op.mult, op.add)
            nc.sync.dma_start(out_d[:, :], out_sb[:])

    nc.compile()
    return nc


def _digit_planes(w):
    """Decompose fp32 weights into ND exact e5m2 digit planes.

    w ~= Wfix * 2^-KBITS with Wfix = sum_i d_i 16^i, d_i in [-8,7].
    Plane i holds d_i * 2^(4i - KBITS + 14); the moving operand carries
    2^-14, so plane_i * moving accumulates to exactly Wfix * 2^-KBITS.
    Returns (planes [ND, *w.shape] e5m2-exact fp32, effective weights fp32).
    """
    wfix = np.round(w.astype(np.float64) * (1 << KBITS)).astype(np.int64)
    assert np.abs(wfix).max() <= DMAX, "weights exceed digit range"
    planes = np.zeros((ND,) + w.shape, np.float32)
    rem = wfix.copy()
    for i in range(ND):
        d = ((rem + 8) % 16) - 8
        rem = (rem - d) >> 4
        planes[i] = d * np.float32(2.0 ** (4 * i - KBITS + 14))
    assert np.all(rem == 0)
    weff = (wfix * (2.0 ** -KBITS)).astype(np.float32)
    return planes, weff


def kernel(input_bins, W_ih, W_hh, W_ho):
    global _BUILT
    if _BUILT is None:
        _BUILT = _build()
    nc = _BUILT
    import ml_dtypes
    e5np = ml_dtypes.float8_e5m2

    input_bins = np.ascontiguousarray(input_bins, dtype=np.float32)
    W_ih = np.ascontiguousarray(W_ih, dtype=np.float32)
    W_hh2 = np.ascontiguousarray(np.asarray(W_hh)[0], dtype=np.float32)
    W_ho = np.ascontiguousarray(W_ho, dtype=np.float32)

    pih, wih_eff = _digit_planes(W_ih)       # [ND, 1024, 2048]
    phh, whh_eff = _digit_planes(W_hh2)      # [ND, 2048, 2048]
    pho, who_eff = _digit_planes(W_ho)       # [ND, 2048, 10]

    # wih planes -> [(m*KI + kt)*128, dig*128]
    wihd = np.ascontiguousarray(
        pih.reshape(ND, KI, 128, KH, 128)      # [dig, kt, p, m, mc]
        .transpose(3, 1, 2, 0, 4)              # [m, kt, p, dig, mc]
        .reshape(KH * INPUT_DIM, ND * 128)
    ).astype(e5np)
    # whh planes -> [(m2*KH + kt)*128, dig*128]
    whhd = np.ascontiguousarray(
        phh.reshape(ND, KH, 128, KH, 128)      # [dig, kt, p, m2, mc]
        .transpose(3, 1, 2, 0, 4)              # [m2, kt, p, dig, mc]
        .reshape(KH * HIDDEN_DIM, ND * 128)
    ).astype(e5np)
    # who planes padded to 16 output cols: [kt*128, dig*16]
    whod = np.zeros((KH, 128, ND, 16), np.float32)
    whod[:, :, :, :OUTPUT_DIM] = pho.reshape(ND, KH, 128, OUTPUT_DIM) \
        .transpose(1, 2, 0, 3)
    whod8 = np.ascontiguousarray(whod.reshape(KH * 128, ND * 16)).astype(e5np)

    cs_hh = np.ascontiguousarray(
        whh_eff.sum(axis=0, dtype=np.float64).astype(np.float32)
        .reshape(KH, 128).T)
    cs_ho = who_eff.sum(axis=0, dtype=np.float64).astype(np.float32) \
        .reshape(OUTPUT_DIM, 1)

    in_maps = []
    for c in range(NCORES):
        xb = input_bins[c * BLOC:(c + 1) * BLOC]        # [32, 1024, 100]
        xc = np.ascontiguousarray(
            xb.transpose(1, 2, 0).reshape(INPUT_DIM, COLS) * np.float32(MOV)
        ).astype(e5np)
        in_maps.append({
            "x": xc, "wihd": wihd, "whhd": whhd, "whod": whod8,
            "cs_hh": cs_hh, "cs_ho": cs_ho,
            "ident": np.eye(128, dtype=np.float32),
        })

    from concourse.bass_utils import run_bass_kernel_spmd
    res = run_bass_kernel_spmd(nc, in_maps, core_ids=list(range(NCORES)))

    out = np.empty((BATCH, OUTPUT_DIM), dtype=np.float32)
    for c in range(NCORES):
        out[c * BLOC:(c + 1) * BLOC] = res.results[c]["out"].T
    return out


# revision 32
# speedup vs baseline: 1.0852x; 1.0237x over previous
"""Trainium2 Bass kernel for a 2-hidden-layer LIF spiking network.

Math (per timestep t, per layer):
    v = 0.9*y + cur ;  spike s = (v >= 1) ;  y = v*(1-s) = v*u  with u = (v < 1)
Layer currents:
    cur1 = x_t @ W_ih            (x binary, precomputable for ALL t)
    cur2 = s1 @ W_hh = colsum(W_hh) - u1 @ W_hh
    cur3 = s2 @ W_ho = colsum(W_ho) - u2 @ W_ho
Output: rate = mean_t s_out = 1 - sum_t(u_out)/T

Key restructurings:
  * Layer 1's recurrence does not depend on layer 2, so all three matmuls are
    batched over the full (T*B) column space; only the cheap elementwise LIF
    scans are sequential in t.
  * Weights are quantized to 23-bit fixed point (step 2^-23) and decomposed
    into ND=6 exact signed base-16 digit planes, each stored in fp8 e5m2
    (digits in [-8,7] and power-of-2 scales are exact in e5m2). The moving
    operands (x and the spike complements u) carry the value 2^-14, exactly
    representable as the e5m2 minimum normal. Pairs of digit planes feed
    fp8 DoubleRow matmuls (2 stationary planes per instruction at 0.5
    cycles/row), so full 23-bit weight precision streams at 1.5 cycles/row
    vs 2.0 for an fp16 hi/lo pair -- with every product exact in fp32 PSUM.
  * The moving AP broadcasts the same spike tile across the DoubleRow pair
    (middle dim stride 0), so spikes are stored once, in SBUF, uncompressed
    (1 byte): the layer-1 spike complement for ALL timesteps stays resident
    and is never spilled to DRAM.

Sharding: data-parallel over batch (256/8 = 32 rows per core), weights
replicated, no cross-core communication.

Per-core schedule:
  Phase A (W_ih digit planes resident, 12.6MB, loaded progressively per
    output chunk so block 0 starts ~2.5us in): mm1 over blocks of 10 steps,
    LIF1 scan fused per block (v computed in-place over cur1), u1 written
    straight into its resident SBUF tile.
  Phase B (W_hh digit planes streamed from DRAM per 128-col output chunk,
    double-buffered): superblocks of 20 steps (last one split 2x10 so the
    final scan hides under mm2); mm2 -> cur2 with colsum correction fused
    into the PSUM->SBUF Identity-activation copy (scale=-1, bias=colsum),
    LIF2 scan, mm3 (emitted mid-way through the next superblock's mm2 so
    the PE never waits on the DVE scan), output LIF scan, final rate.
"""

import numpy as np

# ---- problem constants (hardcoded; kernel.py must be self-contained) ----
BATCH = 256
INPUT_DIM = 1024
HIDDEN_DIM = 2048
OUTPUT_DIM = 10
T = 100
NCORES = 8
BLOC = BATCH // NCORES          # 32 batch rows per core
TB = 5                          # timesteps per phase-A block
NBLK = T // TB                  # 20 blocks
CA = TB * BLOC                  # 160 columns per phase-A block
C = 320                         # columns per phase-B matmul chunk
COLS = T * BLOC                 # 3200 total columns
# phase-B superblocks (col0); u2 for super s lives in spike-buffer slot
# USLOT[s] (slot s-1 is dead once mm2(s) is done; slot 5 is spare for s=0)
SCMAX = 640
SUPERS = [0, 640, 1280, 1920, 2560]
USLOT = [5, 0, 1, 2, 3]
NSLOT = 6
KI = INPUT_DIM // 128           # 8 k-chunks for mm1
KH = HIDDEN_DIM // 128          # 16 k-chunks (and m-chunks) for mm2
DECAY = 0.9
THRESH = 1.0
TH_NUDGE = 0.0                  # tie-break re-roll knob (harmless ~1e-6 scale)

ND = 6                          # digit planes (23-bit fixed point)
KBITS = 23                      # weight step 2^-KBITS (max digit range 7829367)
MOV = 2.0 ** -14                # moving-operand value (e5m2 min normal)
DMAX = 7 * (16 ** ND - 1) // 15

_BUILT = None


def _build():
    """Trace + compile the Bass program once."""
    from contextlib import ExitStack

    import concourse.bacc as bacc
    import concourse.tile as tile
    from concourse import mybir
    from concourse.alu_op_type import AluOpType as op

    f32 = mybir.dt.float32
    e5 = mybir.dt.float8e5
    DR = mybir.MatmulPerfMode.DoubleRow
    ident = mybir.ActivationFunctionType.Identity
    TH = THRESH + TH_NUDGE

    nc = bacc.Bacc("TRN2", target_bir_lowering=False, debug=False,
                   num_devices=NCORES)

    # x values {0, 2^-14}: [input_dim, t*b] t-major columns
    x_d = nc.dram_tensor("x", [INPUT_DIM, COLS], e5,
                         kind="ExternalInput").ap()
    # wih digit planes, m-chunk major: [(m*KI + kt)*128, dig*128]
    wih_d = nc.dram_tensor("wihd", [KH * INPUT_DIM, ND * 128], e5,
                           kind="ExternalInput").ap()
    # whh digit planes: [(m2*KH + kt)*128, dig*128]
    whh_d = nc.dram_tensor("whhd", [KH * HIDDEN_DIM, ND * 128], e5,
                           kind="ExternalInput").ap()
    # who planes padded to 16 cols: [kt*128, dig*16]
    who_d = nc.dram_tensor("whod", [KH * 128, ND * 16], e5,
                           kind="ExternalInput").ap()
    cs_hh_d = nc.dram_tensor("cs_hh", [128, KH], f32, kind="ExternalInput").ap()
    cs_ho_d = nc.dram_tensor("cs_ho", [OUTPUT_DIM, 1], f32,
                             kind="ExternalInput").ap()
    id_d = nc.dram_tensor("ident", [128, 128], f32, kind="ExternalInput").ap()
    out_d = nc.dram_tensor("out", [OUTPUT_DIM, BLOC], f32,
                           kind="ExternalOutput").ap()

    with tile.TileContext(nc) as tc, ExitStack() as ctx:
        # spike complements {0, 2^-14}, resident across both phases
        # [p, kt, col]: cols 0..3200 hold u1; 6 ring slots of 640 also serve
        # as u2 staging (a slot is reused once mm2 has consumed its u1 cols)
        u1_pool = ctx.enter_context(tc.tile_pool(name="u1", bufs=1))
        u1 = u1_pool.tile([128, KH * NSLOT * SCMAX], e5, tag="u1")
        u1_3 = u1[:].rearrange("p (k c) -> p k c", k=KH)

        # ---------------- Phase A: mm1 + LIF1 scan ----------------
        with tc.tile_pool(name="wih", bufs=1) as wih_pool, \
             tc.tile_pool(name="xin", bufs=2) as x_pool, \
             tc.tile_pool(name="cur1", bufs=2) as cur1_pool, \
             tc.tile_pool(name="st1", bufs=1) as st1_pool, \
             tc.tile_pool(name="psA", bufs=6, space="PSUM") as psA:

            # x for blocks 0+1 first so mm1 only ever waits on wih planes
            x_first = x_pool.tile([128, KI * CA], e5, tag="x")
            xf_3 = x_first[:].rearrange("p (k c) -> p k c", k=KI)
            x_b1 = x_pool.tile([128, KI * CA], e5, tag="x")
            xb1_3 = x_b1[:].rearrange("p (k c) -> p k c", k=KI)
            for k in range(KI):
                nc.sync.dma_start(xf_3[:, k, :], x_d[k * 128:(k + 1) * 128,
                                                     0:CA])
                nc.sync.dma_start(xb1_3[:, k, :],
                                  x_d[k * 128:(k + 1) * 128, CA:2 * CA])
            # wih digit planes, per m-chunk (progressive: mm1 m-chunk can
            # start as soon as its planes land)
            wih_sb = []
            for m in range(KH):
                w = wih_pool.tile([128, KI * ND * 128], e5, tag=f"wih_{m}")
                nc.sync.dma_start(
                    w[:].rearrange("p (k f) -> p k f", k=KI),
                    wih_d[m * INPUT_DIM:(m + 1) * INPUT_DIM, :]
                    .rearrange("(k p) f -> p k f", p=128))
                wih_sb.append(w)

            y1 = st1_pool.tile([128, KH * BLOC], f32, tag="y1")
            nc.vector.memset(y1[:], 0.0)
            y1_3 = y1[:].rearrange("p (m b) -> p m b", m=KH)

            def mm1_block(xt_3, cur1, m):
                ps = psA.tile([128, CA], f32, tag="psA")
                w4 = wih_sb[m][:].rearrange("p (k i m) -> p k i m",
                                            k=KI, i=ND)
                for k in range(KI):
                    xb = xt_3[:, k, :].unsqueeze(1) \
                        .broadcast_to([128, 2, CA])
                    for j in range(ND // 2):
                        nc.tensor.matmul(
                            ps[:],
                            w4[:, k, 2 * j:2 * j + 2, :],
                            xb,
                            start=(k == 0 and j == 0),
                            stop=(k == KI - 1 and j == ND // 2 - 1),
                            perf_mode=DR)
                nc.scalar.copy(cur1[:, m * CA:(m + 1) * CA], ps[:])

            def scan1_block(cur1, c0):
                cur1_r = cur1[:].rearrange("p (m c) -> p m c", m=KH)
                for t in range(TB):
                    # v computed in-place over the cur1 slice
                    v = cur1_r[:, :, t * BLOC:(t + 1) * BLOC]
                    ub = u1_3[:, :, c0 + t * BLOC:c0 + (t + 1) * BLOC]
                    # v = 0.9*y + cur
                    nc.vector.scalar_tensor_tensor(v, y1_3, DECAY, v,
                                                   op.mult, op.add)
                    # u = (v < 1) * 2^-14, e5m2 for the DoubleRow matmul
                    nc.vector.tensor_scalar(ub, v, TH, MOV,
                                            op.is_lt, op.mult)
                    # y = (v<1)*v
                    nc.vector.scalar_tensor_tensor(y1_3, v, TH, v,
                                                   op.is_lt, op.mult)

            # blocks 0+1 interleaved per m-chunk: mm1 rides the progressive
            # wih DMA (one m-chunk's planes feed both blocks back to back)
            cur1_b0 = cur1_pool.tile([128, KH * CA], f32, tag="cur1")
            cur1_b1 = cur1_pool.tile([128, KH * CA], f32, tag="cur1")
            for m in range(KH):
                mm1_block(xf_3, cur1_b0, m)
                mm1_block(xb1_3, cur1_b1, m)
            scan1_block(cur1_b0, 0)
            scan1_block(cur1_b1, CA)

            for blk in range(2, NBLK):
                c0 = blk * CA
                xt = x_pool.tile([128, KI * CA], e5, tag="x")
                xt_3 = xt[:].rearrange("p (k c) -> p k c", k=KI)
                for k in range(KI):
                    nc.sync.dma_start(xt_3[:, k, :],
                                      x_d[k * 128:(k + 1) * 128,
                                          c0:c0 + CA])
                cur1 = cur1_pool.tile([128, KH * CA], f32, tag="cur1")
                for m in range(KH):
                    mm1_block(xt_3, cur1, m)
                scan1_block(cur1, c0)

        # ---------------- Phase B: mm2 + LIF2 + mm3 + output scan -----------
        with tc.tile_pool(name="wst", bufs=3) as wst_pool, \
             tc.tile_pool(name="cur2", bufs=2) as cur2_pool, \
             tc.tile_pool(name="smallB", bufs=1) as sm_pool, \
             tc.tile_pool(name="cur3", bufs=1) as cur3_pool, \
             tc.tile_pool(name="s3p", bufs=2) as s3_pool, \
             tc.tile_pool(name="psB", bufs=4, space="PSUM") as psB, \
             tc.tile_pool(name="ps3", bufs=2, space="PSUM") as ps3_pool, \
             tc.tile_pool(name="pstr", bufs=2, space="PSUM") as pstr_pool:

            # first two whh chunks ahead of the small constant loads: they
            # gate the first mm2
            wst_head = []
            for m2 in range(2):
                wst = wst_pool.tile([128, KH * ND * 128], e5, tag="wst")
                nc.sync.dma_start(
                    wst[:].rearrange("p (k f) -> p k f", k=KH),
                    whh_d[m2 * HIDDEN_DIM:(m2 + 1) * HIDDEN_DIM, :]
                    .rearrange("(k p) f -> p k f", p=128))
                wst_head.append(wst)

            who_sb = sm_pool.tile([128, KH * ND * 16], e5, tag="who")
            nc.sync.dma_start(
                who_sb[:].rearrange("p (k f) -> p k f", k=KH),
                who_d[:, :].rearrange("(k p) f -> p k f", p=128))
            who4 = who_sb[:].rearrange("p (k i m) -> p k i m", k=KH, i=ND)
            cs_hh = sm_pool.tile([128, KH], f32, tag="cshh")
            nc.sync.dma_start(cs_hh[:], cs_hh_d[:, :])
            cs_ho = sm_pool.tile([OUTPUT_DIM, 1], f32, tag="csho")
            nc.sync.dma_start(cs_ho[:], cs_ho_d[:, :])
            ident_sb = sm_pool.tile([128, 128], f32, tag="ident")
            nc.sync.dma_start(ident_sb[:], id_d[:, :])

            y2 = sm_pool.tile([128, KH * BLOC], f32, tag="y2")
            yo = sm_pool.tile([OUTPUT_DIM, BLOC], f32, tag="yo")
            vo = sm_pool.tile([OUTPUT_DIM, BLOC], f32, tag="vo")
            acc0 = sm_pool.tile([OUTPUT_DIM, BLOC], f32, tag="acc0")
            acc1 = sm_pool.tile([OUTPUT_DIM, BLOC], f32, tag="acc1")
            acc = [acc0, acc1]
            out_sb = sm_pool.tile([OUTPUT_DIM, BLOC], f32, tag="rate")
            nc.vector.memset(y2[:], 0.0)
            nc.vector.memset(yo[:], 0.0)
            nc.vector.memset(acc[0][:], 0.0)
            y2_3 = y2[:].rearrange("p (m b) -> p m b", m=KH)

            def emit_mm3(c0, uoff):
                """mm3 (transposed: moving side = W_ho planes, 10-wide
                output) + PE transpose back + output-layer scan."""
                cur3 = cur3_pool.tile([OUTPUT_DIM, SCMAX], f32, tag="cur3")
                for ch in range(SCMAX // 128):
                    ps3 = ps3_pool.tile([128, OUTPUT_DIM], f32, tag="ps3")
                    for k in range(KH // 2):
                        # stationary: u2 k-tile pair; moving: W_ho planes
                        ub = u1_3[:, 2 * k:2 * k + 2,
                                  uoff + ch * 128:uoff + (ch + 1) * 128]
                        for i in range(ND):
                            nc.tensor.matmul(
                                ps3[:],
                                ub,
                                who4[:, 2 * k:2 * k + 2, i,
                                     0:OUTPUT_DIM],
                                start=(k == 0 and i == 0),
                                stop=(k == KH // 2 - 1 and i == ND - 1),
                                perf_mode=DR)
                    s3 = s3_pool.tile([128, OUTPUT_DIM], f32, tag="s3")
                    nc.scalar.copy(s3[:], ps3[:])
                    pst = pstr_pool.tile([OUTPUT_DIM, 128], f32, tag="pst")
                    nc.tensor.transpose(pst[:], s3[:], ident_sb[:])
                    # cur3 = colsum_ho - u2@W_ho  (true output current)
                    nc.scalar.activation(cur3[:, ch * 128:(ch + 1) * 128],
                                         pst[:], ident,
                                         bias=cs_ho[:, 0:1], scale=-1.0)
                for t in range(SCMAX // BLOC):
                    g = c0 // BLOC + t
                    sl = cur3[:, t * BLOC:(t + 1) * BLOC]
                    nc.vector.scalar_tensor_tensor(vo[:], yo[:], DECAY, sl,
                                                   op.mult, op.add)
                    nc.vector.scalar_tensor_tensor(acc[(g + 1) % 2][:], vo[:],
                                                   TH, acc[g % 2][:],
                                                   op.is_lt, op.add)
                    nc.vector.scalar_tensor_tensor(yo[:], vo[:], TH, vo[:],
                                                   op.is_lt, op.mult)

            prev = None
            for sup, c0 in enumerate(SUPERS):
                uoff = USLOT[sup] * SCMAX
                cur2 = cur2_pool.tile([128, KH * SCMAX], f32, tag="cur2")
                for m2 in range(KH):
                    if m2 == 8 and prev is not None:
                        emit_mm3(*prev)
                        prev = None
                    if sup == 0 and m2 < 2:
                        wst = wst_head[m2]
                    else:
                        wst = wst_pool.tile([128, KH * ND * 128], e5,
                                            tag="wst")
                        nc.sync.dma_start(
                            wst[:].rearrange("p (k f) -> p k f", k=KH),
                            whh_d[m2 * HIDDEN_DIM:(m2 + 1) * HIDDEN_DIM, :]
                            .rearrange("(k p) f -> p k f", p=128))
                    wst4 = wst[:].rearrange("p (k i m) -> p k i m",
                                            k=KH, i=ND)
                    for nh in range(SCMAX // C):
                        ps = psB.tile([128, C], f32, tag="psB")
                        for k in range(KH):
                            ub = u1_3[:, k,
                                      c0 + nh * C:c0 + (nh + 1) * C] \
                                .unsqueeze(1).broadcast_to([128, 2, C])
                            for j in range(ND // 2):
                                nc.tensor.matmul(
                                    ps[:],
                                    wst4[:, k, 2 * j:2 * j + 2, :],
                                    ub,
                                    start=(k == 0 and j == 0),
                                    stop=(k == KH - 1 and j == ND // 2 - 1),
                                    perf_mode=DR)
                        # cur2 = colsum_hh - u1@W_hh (true layer-2 current)
                        nc.scalar.activation(
                            cur2[:,
                                 m2 * SCMAX + nh * C:m2 * SCMAX + (nh + 1) * C],
                            ps[:], ident, bias=cs_hh[:, m2:m2 + 1],
                            scale=-1.0)
                cur2_r = cur2[:].rearrange("p (m c) -> p m c", m=KH)
                parts = [(nc.vector, 0, KH)]
                for t in range(SCMAX // BLOC):
                    for eng, ml, mh in parts:
                        # v computed in-place over the cur2 slice
                        v = cur2_r[:, ml:mh, t * BLOC:(t + 1) * BLOC]
                        ub = u1_3[:, ml:mh,
                                  uoff + t * BLOC:uoff + (t + 1) * BLOC]
                        yy = y2_3[:, ml:mh, :]
                        eng.scalar_tensor_tensor(v, yy, DECAY, v,
                                                 op.mult, op.add)
                        eng.tensor_scalar(ub, v, TH, MOV,
                                          op.is_lt, op.mult)
                        eng.scalar_tensor_tensor(yy, v, TH, v,
                                                 op.is_lt, op.mult)
                prev = (c0, uoff)
            emit_mm3(*prev)

            # rate = 1 - acc/T   (acc holds sum of u_out; s = 1-u)
            nc.vector.tensor_scalar(out_sb[:], acc[T % 2][:], -1.0 / T, 1.0,
                                    op.mult, op.add)
            nc.sync.dma_start(out_d[:, :], out_sb[:])

    nc.compile()
    return nc


def _digit_planes(w):
    """Decompose fp32 weights into ND exact e5m2 digit planes.

    w ~= Wfix * 2^-KBITS with Wfix = sum_i d_i 16^i, d_i in [-8,7].
    Plane i holds d_i * 2^(4i - KBITS + 14); the moving operand carries
    2^-14, so plane_i * moving accumulates to exactly Wfix * 2^-KBITS.
    Returns (planes [ND, *w.shape] e5m2-exact fp32, effective weights fp32).
    """
    wfix = np.round(w.astype(np.float64) * (1 << KBITS)).astype(np.int64)
    assert np.abs(wfix).max() <= DMAX, "weights exceed digit range"
    planes = np.zeros((ND,) + w.shape, np.float32)
    rem = wfix.copy()
    for i in range(ND):
        d = ((rem + 8) % 16) - 8
        rem = (rem - d) >> 4
        planes[i] = d * np.float32(2.0 ** (4 * i - KBITS + 14))
    assert np.all(rem == 0)
    weff = (wfix * (2.0 ** -KBITS)).astype(np.float32)
    return planes, weff


def kernel(input_bins, W_ih, W_hh, W_ho):
    global _BUILT
    if _BUILT is None:
        _BUILT = _build()
    nc = _BUILT
    import ml_dtypes
    e5np = ml_dtypes.float8_e5m2

    input_bins = np.ascontiguousarray(input_bins, dtype=np.float32)
    W_ih = np.ascontiguousarray(W_ih, dtype=np.float32)
    W_hh2 = np.ascontiguousarray(np.asarray(W_hh)[0], dtype=np.float32)
    W_ho = np.ascontiguousarray(W_ho, dtype=np.float32)

    pih, wih_eff = _digit_planes(W_ih)       # [ND, 1024, 2048]
    phh, whh_eff = _digit_planes(W_hh2)      # [ND, 2048, 2048]
    pho, who_eff = _digit_planes(W_ho)       # [ND, 2048, 10]

    # wih planes -> [(m*KI + kt)*128, dig*128]
    wihd = np.ascontiguousarray(
        pih.reshape(ND, KI, 128, KH, 128)      # [dig, kt, p, m, mc]
        .transpose(3, 1, 2, 0, 4)              # [m, kt, p, dig, mc]
        .reshape(KH * INPUT_DIM, ND * 128)
    ).astype(e5np)
    # whh planes -> [(m2*KH + kt)*128, dig*128]
    whhd = np.ascontiguousarray(
        phh.reshape(ND, KH, 128, KH, 128)      # [dig, kt, p, m2, mc]
        .transpose(3, 1, 2, 0, 4)              # [m2, kt, p, dig, mc]
        .reshape(KH * HIDDEN_DIM, ND * 128)
    ).astype(e5np)
    # who planes padded to 16 output cols: [kt*128, dig*16]
    whod = np.zeros((KH, 128, ND, 16), np.float32)
    whod[:, :, :, :OUTPUT_DIM] = pho.reshape(ND, KH, 128, OUTPUT_DIM) \
        .transpose(1, 2, 0, 3)
    whod8 = np.ascontiguousarray(whod.reshape(KH * 128, ND * 16)).astype(e5np)

    cs_hh = np.ascontiguousarray(
        whh_eff.sum(axis=0, dtype=np.float64).astype(np.float32)
        .reshape(KH, 128).T)
    cs_ho = who_eff.sum(axis=0, dtype=np.float64).astype(np.float32) \
        .reshape(OUTPUT_DIM, 1)

    in_maps = []
    for c in range(NCORES):
        xb = input_bins[c * BLOC:(c + 1) * BLOC]        # [32, 1024, 100]
        xc = np.ascontiguousarray(
            xb.transpose(1, 2, 0).reshape(INPUT_DIM, COLS) * np.float32(MOV)
        ).astype(e5np)
        in_maps.append({
            "x": xc, "wihd": wihd, "whhd": whhd, "whod": whod8,
            "cs_hh": cs_hh, "cs_ho": cs_ho,
            "ident": np.eye(128, dtype=np.float32),
        })

    from concourse.bass_utils import run_bass_kernel_spmd
    res = run_bass_kernel_spmd(nc, in_maps, core_ids=list(range(NCORES)))

    out = np.empty((BATCH, OUTPUT_DIM), dtype=np.float32)
    for c in range(NCORES):
        out[c * BLOC:(c + 1) * BLOC] = res.results[c]["out"].T
    return out


# revision 34
# speedup vs baseline: 1.0963x; 1.0102x over previous
"""Trainium2 Bass kernel for a 2-hidden-layer LIF spiking network.

Math (per timestep t, per layer):
    v = 0.9*y + cur ;  spike s = (v >= 1) ;  y = v*(1-s) = v*u  with u = (v < 1)
Layer currents:
    cur1 = x_t @ W_ih            (x binary, precomputable for ALL t)
    cur2 = s1 @ W_hh = colsum(W_hh) - u1 @ W_hh
    cur3 = s2 @ W_ho = colsum(W_ho) - u2 @ W_ho
Output: rate = mean_t s_out = 1 - sum_t(u_out)/T

Key restructurings:
  * Layer 1's recurrence does not depend on layer 2, so all three matmuls are
    batched over the full (T*B) column space; only the cheap elementwise LIF
    scans are sequential in t.
  * Weights are quantized to 23-bit fixed point (step 2^-23) and decomposed
    into ND=6 exact signed base-16 digit planes, each stored in fp8 e5m2
    (digits in [-8,7] and power-of-2 scales are exact in e5m2). The moving
    operands (x and the spike complements u) carry the value 2^-14, exactly
    representable as the e5m2 minimum normal. Pairs of digit planes feed
    fp8 DoubleRow matmuls (2 stationary planes per instruction at 0.5
    cycles/row), so full 23-bit weight precision streams at 1.5 cycles/row
    vs 2.0 for an fp16 hi/lo pair -- with every product exact in fp32 PSUM.
  * The moving AP broadcasts the same spike tile across the DoubleRow pair
    (middle dim stride 0), so spikes are stored once, in SBUF, uncompressed
    (1 byte): the layer-1 spike complement for ALL timesteps stays resident
    and is never spilled to DRAM.

Sharding: data-parallel over batch (256/8 = 32 rows per core), weights
replicated, no cross-core communication.

Per-core schedule:
  Phase A (W_ih digit planes resident, 12.6MB, loaded progressively per
    output chunk so block 0 starts ~2.5us in): mm1 over blocks of 10 steps,
    LIF1 scan fused per block (v computed in-place over cur1), u1 written
    straight into its resident SBUF tile.
  Phase B (W_hh digit planes streamed from DRAM per 128-col output chunk,
    double-buffered): superblocks of 20 steps (last one split 2x10 so the
    final scan hides under mm2); mm2 -> cur2 with colsum correction fused
    into the PSUM->SBUF Identity-activation copy (scale=-1, bias=colsum),
    LIF2 scan, mm3 (emitted mid-way through the next superblock's mm2 so
    the PE never waits on the DVE scan), output LIF scan, final rate.
"""

import numpy as np

# ---- problem constants (hardcoded; kernel.py must be self-contained) ----
BATCH = 256
INPUT_DIM = 1024
HIDDEN_DIM = 2048
OUTPUT_DIM = 10
T = 100
NCORES = 8
BLOC = BATCH // NCORES          # 32 batch rows per core
TB = 5                          # timesteps per phase-A block
NBLK = T // TB                  # 20 blocks
CA = TB * BLOC                  # 160 columns per phase-A block
C = 320                         # columns per phase-B matmul chunk
COLS = T * BLOC                 # 3200 total columns
# phase-B superblocks (col0); u2 for super s lives in spike-buffer slot
# USLOT[s] (slot s-1 is dead once mm2(s) is done; slot 5 is spare for s=0)
SCMAX = 640
SUPERS = [0, 640, 1280, 1920, 2560]
USLOT = [5, 0, 1, 2, 3]
NSLOT = 6
KI = INPUT_DIM // 128           # 8 k-chunks for mm1
KH = HIDDEN_DIM // 128          # 16 k-chunks (and m-chunks) for mm2
DECAY = 0.9
THRESH = 1.0
TH_NUDGE = 0.0                  # tie-break re-roll knob (harmless ~1e-6 scale)

ND = 6                          # digit planes (23-bit fixed point)
KBITS = 23                      # weight step 2^-KBITS (max digit range 7829367)
MOV = 2.0 ** -14                # moving-operand value (e5m2 min normal)
DMAX = 7 * (16 ** ND - 1) // 15

_BUILT = None


def _build():
    """Trace + compile the Bass program once."""
    from contextlib import ExitStack

    import concourse.bacc as bacc
    import concourse.tile as tile
    from concourse import mybir
    from concourse.alu_op_type import AluOpType as op

    f32 = mybir.dt.float32
    e5 = mybir.dt.float8e5
    DR = mybir.MatmulPerfMode.DoubleRow
    ident = mybir.ActivationFunctionType.Identity
    TH = THRESH + TH_NUDGE

    nc = bacc.Bacc("TRN2", target_bir_lowering=False, debug=False,
                   num_devices=NCORES)

    # x values {0, 2^-14}: [input_dim, t*b] t-major columns
    x_d = nc.dram_tensor("x", [INPUT_DIM, COLS], e5,
                         kind="ExternalInput").ap()
    # wih digit planes, m-chunk major: [(m*KI + kt)*128, dig*128]
    wih_d = nc.dram_tensor("wihd", [KH * INPUT_DIM, ND * 128], e5,
                           kind="ExternalInput").ap()
    # whh digit planes: [(m2*KH + kt)*128, dig*128]
    whh_d = nc.dram_tensor("whhd", [KH * HIDDEN_DIM, ND * 128], e5,
                           kind="ExternalInput").ap()
    # who planes padded to 16 cols: [kt*128, dig*16]
    who_d = nc.dram_tensor("whod", [KH * 128, ND * 16], e5,
                           kind="ExternalInput").ap()
    cs_hh_d = nc.dram_tensor("cs_hh", [128, KH], f32, kind="ExternalInput").ap()
    cs_ho_d = nc.dram_tensor("cs_ho", [OUTPUT_DIM, 1], f32,
                             kind="ExternalInput").ap()
    id_d = nc.dram_tensor("ident", [128, 128], f32, kind="ExternalInput").ap()
    out_d = nc.dram_tensor("out", [OUTPUT_DIM, BLOC], f32,
                           kind="ExternalOutput").ap()

    with tile.TileContext(nc) as tc, ExitStack() as ctx:
        # spike complements {0, 2^-14}, resident across both phases
        # [p, kt, col]: cols 0..3200 hold u1; 6 ring slots of 640 also serve
        # as u2 staging (a slot is reused once mm2 has consumed its u1 cols)
        u1_pool = ctx.enter_context(tc.tile_pool(name="u1", bufs=1))
        u1 = u1_pool.tile([128, KH * NSLOT * SCMAX], e5, tag="u1")
        u1_3 = u1[:].rearrange("p (k c) -> p k c", k=KH)

        # ---------------- Phase A: mm1 + LIF1 scan ----------------
        with tc.tile_pool(name="wih", bufs=1) as wih_pool, \
             tc.tile_pool(name="xin", bufs=2) as x_pool, \
             tc.tile_pool(name="cur1", bufs=2) as cur1_pool, \
             tc.tile_pool(name="st1", bufs=1) as st1_pool, \
             tc.tile_pool(name="psA", bufs=6, space="PSUM") as psA:

            def load_x(xt_3, c0):
                nc.sync.dma_start(
                    xt_3,
                    x_d[:, c0:c0 + CA].rearrange("(k p) c -> p k c", p=128))

            # x for blocks 0+1 first so mm1 only ever waits on wih planes
            x_first = x_pool.tile([128, KI * CA], e5, tag="x")
            xf_3 = x_first[:].rearrange("p (k c) -> p k c", k=KI)
            x_b1 = x_pool.tile([128, KI * CA], e5, tag="x")
            xb1_3 = x_b1[:].rearrange("p (k c) -> p k c", k=KI)
            load_x(xf_3, 0)
            load_x(xb1_3, CA)
            # wih digit planes, per m-chunk (progressive: mm1 m-chunk can
            # start as soon as its planes land)
            wih_sb = []
            for m in range(KH):
                w = wih_pool.tile([128, KI * ND * 128], e5, tag=f"wih_{m}")
                nc.sync.dma_start(
                    w[:].rearrange("p (k f) -> p k f", k=KI),
                    wih_d[m * INPUT_DIM:(m + 1) * INPUT_DIM, :]
                    .rearrange("(k p) f -> p k f", p=128))
                wih_sb.append(w)

            y1 = st1_pool.tile([128, KH * BLOC], f32, tag="y1")
            nc.vector.memset(y1[:], 0.0)
            y1_3 = y1[:].rearrange("p (m b) -> p m b", m=KH)

            def mm1_block(xt_3, cur1, m):
                ps = psA.tile([128, CA], f32, tag="psA")
                w4 = wih_sb[m][:].rearrange("p (k i m) -> p k i m",
                                            k=KI, i=ND)
                for k in range(KI):
                    xb = xt_3[:, k, :].unsqueeze(1) \
                        .broadcast_to([128, 2, CA])
                    for j in range(ND // 2):
                        nc.tensor.matmul(
                            ps[:],
                            w4[:, k, 2 * j:2 * j + 2, :],
                            xb,
                            start=(k == 0 and j == 0),
                            stop=(k == KI - 1 and j == ND // 2 - 1),
                            perf_mode=DR)
                nc.scalar.copy(cur1[:, m * CA:(m + 1) * CA], ps[:])

            def scan1_block(cur1, c0):
                cur1_r = cur1[:].rearrange("p (m c) -> p m c", m=KH)
                for t in range(TB):
                    # v computed in-place over the cur1 slice
                    v = cur1_r[:, :, t * BLOC:(t + 1) * BLOC]
                    ub = u1_3[:, :, c0 + t * BLOC:c0 + (t + 1) * BLOC]
                    # v = 0.9*y + cur
                    nc.vector.scalar_tensor_tensor(v, y1_3, DECAY, v,
                                                   op.mult, op.add)
                    # u = (v < 1) * 2^-14, e5m2 for the DoubleRow matmul
                    nc.vector.tensor_scalar(ub, v, TH, MOV,
                                            op.is_lt, op.mult)
                    # y = (v<1)*v
                    nc.vector.scalar_tensor_tensor(y1_3, v, TH, v,
                                                   op.is_lt, op.mult)

            # blocks 0+1 interleaved per m-chunk: mm1 rides the progressive
            # wih DMA (one m-chunk's planes feed both blocks back to back)
            cur1_b0 = cur1_pool.tile([128, KH * CA], f32, tag="cur1")
            cur1_b1 = cur1_pool.tile([128, KH * CA], f32, tag="cur1")
            for m in range(KH):
                mm1_block(xf_3, cur1_b0, m)
                mm1_block(xb1_3, cur1_b1, m)
            scan1_block(cur1_b0, 0)
            scan1_block(cur1_b1, CA)

            for blk in range(2, NBLK):
                c0 = blk * CA
                xt = x_pool.tile([128, KI * CA], e5, tag="x")
                xt_3 = xt[:].rearrange("p (k c) -> p k c", k=KI)
                load_x(xt_3, c0)
                cur1 = cur1_pool.tile([128, KH * CA], f32, tag="cur1")
                for m in range(KH):
                    mm1_block(xt_3, cur1, m)
                scan1_block(cur1, c0)

        # ---------------- Phase B: mm2 + LIF2 + mm3 + output scan -----------
        with tc.tile_pool(name="wst", bufs=3) as wst_pool, \
             tc.tile_pool(name="cur2", bufs=2) as cur2_pool, \
             tc.tile_pool(name="smallB", bufs=1) as sm_pool, \
             tc.tile_pool(name="cur3", bufs=1) as cur3_pool, \
             tc.tile_pool(name="s3p", bufs=2) as s3_pool, \
             tc.tile_pool(name="psB", bufs=4, space="PSUM") as psB, \
             tc.tile_pool(name="ps3", bufs=2, space="PSUM") as ps3_pool, \
             tc.tile_pool(name="pstr", bufs=2, space="PSUM") as pstr_pool:

            # first two whh chunks ahead of the small constant loads: they
            # gate the first mm2
            wst_head = []
            for m2 in range(2):
                wst = wst_pool.tile([128, KH * ND * 128], e5, tag="wst")
                nc.sync.dma_start(
                    wst[:].rearrange("p (k f) -> p k f", k=KH),
                    whh_d[m2 * HIDDEN_DIM:(m2 + 1) * HIDDEN_DIM, :]
                    .rearrange("(k p) f -> p k f", p=128))
                wst_head.append(wst)

            who_sb = sm_pool.tile([128, KH * ND * 16], e5, tag="who")
            nc.sync.dma_start(
                who_sb[:].rearrange("p (k f) -> p k f", k=KH),
                who_d[:, :].rearrange("(k p) f -> p k f", p=128))
            who4 = who_sb[:].rearrange("p (k i m) -> p k i m", k=KH, i=ND)
            cs_hh = sm_pool.tile([128, KH], f32, tag="cshh")
            nc.sync.dma_start(cs_hh[:], cs_hh_d[:, :])
            cs_ho = sm_pool.tile([OUTPUT_DIM, 1], f32, tag="csho")
            nc.sync.dma_start(cs_ho[:], cs_ho_d[:, :])
            ident_sb = sm_pool.tile([128, 128], f32, tag="ident")
            nc.sync.dma_start(ident_sb[:], id_d[:, :])

            y2 = sm_pool.tile([128, KH * BLOC], f32, tag="y2")
            yo = sm_pool.tile([OUTPUT_DIM, BLOC], f32, tag="yo")
            vo = sm_pool.tile([OUTPUT_DIM, BLOC], f32, tag="vo")
            acc0 = sm_pool.tile([OUTPUT_DIM, BLOC], f32, tag="acc0")
            acc1 = sm_pool.tile([OUTPUT_DIM, BLOC], f32, tag="acc1")
            acc = [acc0, acc1]
            out_sb = sm_pool.tile([OUTPUT_DIM, BLOC], f32, tag="rate")
            nc.vector.memset(y2[:], 0.0)
            nc.vector.memset(yo[:], 0.0)
            nc.vector.memset(acc[0][:], 0.0)
            y2_3 = y2[:].rearrange("p (m b) -> p m b", m=KH)

            def emit_mm3(c0, uoff):
                """mm3 (transposed: moving side = W_ho planes, 10-wide
                output) + PE transpose back + output-layer scan."""
                cur3 = cur3_pool.tile([OUTPUT_DIM, SCMAX], f32, tag="cur3")
                for ch in range(SCMAX // 128):
                    ps3 = ps3_pool.tile([128, OUTPUT_DIM], f32, tag="ps3")
                    for k in range(KH // 2):
                        # stationary: u2 k-tile pair; moving: W_ho planes
                        ub = u1_3[:, 2 * k:2 * k + 2,
                                  uoff + ch * 128:uoff + (ch + 1) * 128]
                        for i in range(ND):
                            nc.tensor.matmul(
                                ps3[:],
                                ub,
                                who4[:, 2 * k:2 * k + 2, i,
                                     0:OUTPUT_DIM],
                                start=(k == 0 and i == 0),
                                stop=(k == KH // 2 - 1 and i == ND - 1),
                                perf_mode=DR)
                    s3 = s3_pool.tile([128, OUTPUT_DIM], f32, tag="s3")
                    nc.scalar.copy(s3[:], ps3[:])
                    pst = pstr_pool.tile([OUTPUT_DIM, 128], f32, tag="pst")
                    nc.tensor.transpose(pst[:], s3[:], ident_sb[:])
                    # cur3 = colsum_ho - u2@W_ho  (true output current)
                    nc.scalar.activation(cur3[:, ch * 128:(ch + 1) * 128],
                                         pst[:], ident,
                                         bias=cs_ho[:, 0:1], scale=-1.0)
                for t in range(SCMAX // BLOC):
                    g = c0 // BLOC + t
                    sl = cur3[:, t * BLOC:(t + 1) * BLOC]
                    nc.vector.scalar_tensor_tensor(vo[:], yo[:], DECAY, sl,
                                                   op.mult, op.add)
                    nc.vector.scalar_tensor_tensor(acc[(g + 1) % 2][:], vo[:],
                                                   TH, acc[g % 2][:],
                                                   op.is_lt, op.add)
                    nc.vector.scalar_tensor_tensor(yo[:], vo[:], TH, vo[:],
                                                   op.is_lt, op.mult)

            prev = None
            for sup, c0 in enumerate(SUPERS):
                uoff = USLOT[sup] * SCMAX
                cur2 = cur2_pool.tile([128, KH * SCMAX], f32, tag="cur2")
                for m2 in range(KH):
                    if m2 == 8 and prev is not None:
                        emit_mm3(*prev)
                        prev = None
                    if sup == 0 and m2 < 2:
                        wst = wst_head[m2]
                    else:
                        wst = wst_pool.tile([128, KH * ND * 128], e5,
                                            tag="wst")
                        nc.sync.dma_start(
                            wst[:].rearrange("p (k f) -> p k f", k=KH),
                            whh_d[m2 * HIDDEN_DIM:(m2 + 1) * HIDDEN_DIM, :]
                            .rearrange("(k p) f -> p k f", p=128))
                    wst4 = wst[:].rearrange("p (k i m) -> p k i m",
                                            k=KH, i=ND)
                    for nh in range(SCMAX // C):
                        ps = psB.tile([128, C], f32, tag="psB")
                        for k in range(KH):
                            ub = u1_3[:, k,
                                      c0 + nh * C:c0 + (nh + 1) * C] \
                                .unsqueeze(1).broadcast_to([128, 2, C])
                            for j in range(ND // 2):
                                nc.tensor.matmul(
                                    ps[:],
                                    wst4[:, k, 2 * j:2 * j + 2, :],
                                    ub,
                                    start=(k == 0 and j == 0),
                                    stop=(k == KH - 1 and j == ND // 2 - 1),
                                    perf_mode=DR)
                        # cur2 = colsum_hh - u1@W_hh (true layer-2 current)
                        nc.scalar.activation(
                            cur2[:,
                                 m2 * SCMAX + nh * C:m2 * SCMAX + (nh + 1) * C],
                            ps[:], ident, bias=cs_hh[:, m2:m2 + 1],
                            scale=-1.0)
                cur2_r = cur2[:].rearrange("p (m c) -> p m c", m=KH)
                parts = [(nc.vector, 0, KH)]
                for t in range(SCMAX // BLOC):
                    for eng, ml, mh in parts:
                        # v computed in-place over the cur2 slice
                        v = cur2_r[:, ml:mh, t * BLOC:(t + 1) * BLOC]
                        ub = u1_3[:, ml:mh,
                                  uoff + t * BLOC:uoff + (t + 1) * BLOC]
                        yy = y2_3[:, ml:mh, :]
                        eng.scalar_tensor_tensor(v, yy, DECAY, v,
                                                 op.mult, op.add)
                        eng.tensor_scalar(ub, v, TH, MOV,
                                          op.is_lt, op.mult)
                        eng.scalar_tensor_tensor(yy, v, TH, v,
                                                 op.is_lt, op.mult)
                prev = (c0, uoff)
            emit_mm3(*prev)

            # rate = 1 - acc/T   (acc holds sum of u_out; s = 1-u)
            nc.vector.tensor_scalar(out_sb[:], acc[T % 2][:], -1.0 / T, 1.0,
                                    op.mult, op.add)
            nc.sync.dma_start(out_d[:, :], out_sb[:])

    nc.compile()
    return nc


def _digit_planes(w):
    """Decompose fp32 weights into ND exact e5m2 digit planes.

    w ~= Wfix * 2^-KBITS with Wfix = sum_i d_i 16^i, d_i in [-8,7].
    Plane i holds d_i * 2^(4i - KBITS + 14); the moving operand carries
    2^-14, so plane_i * moving accumulates to exactly Wfix * 2^-KBITS.
    Returns (planes [ND, *w.shape] e5m2-exact fp32, effective weights fp32).
    """
    wfix = np.round(w.astype(np.float64) * (1 << KBITS)).astype(np.int64)
    assert np.abs(wfix).max() <= DMAX, "weights exceed digit range"
    planes = np.zeros((ND,) + w.shape, np.float32)
    rem = wfix.copy()
    for i in range(ND):
        d = ((rem + 8) % 16) - 8
        rem = (rem - d) >> 4
        planes[i] = d * np.float32(2.0 ** (4 * i - KBITS + 14))
    assert np.all(rem == 0)
    weff = (wfix * (2.0 ** -KBITS)).astype(np.float32)
    return planes, weff


def kernel(input_bins, W_ih, W_hh, W_ho):
    global _BUILT
    if _BUILT is None:
        _BUILT = _build()
    nc = _BUILT
    import ml_dtypes
    e5np = ml_dtypes.float8_e5m2

    input_bins = np.ascontiguousarray(input_bins, dtype=np.float32)
    W_ih = np.ascontiguousarray(W_ih, dtype=np.float32)
    W_hh2 = np.ascontiguousarray(np.asarray(W_hh)[0], dtype=np.float32)
    W_ho = np.ascontiguousarray(W_ho, dtype=np.float32)

    pih, wih_eff = _digit_planes(W_ih)       # [ND, 1024, 2048]
    phh, whh_eff = _digit_planes(W_hh2)      # [ND, 2048, 2048]
    pho, who_eff = _digit_planes(W_ho)       # [ND, 2048, 10]

    # wih planes -> [(m*KI + kt)*128, dig*128]
    wihd = np.ascontiguousarray(
        pih.reshape(ND, KI, 128, KH, 128)      # [dig, kt, p, m, mc]
        .transpose(3, 1, 2, 0, 4)              # [m, kt, p, dig, mc]
        .reshape(KH * INPUT_DIM, ND * 128)
    ).astype(e5np)
    # whh planes -> [(m2*KH + kt)*128, dig*128]
    whhd = np.ascontiguousarray(
        phh.reshape(ND, KH, 128, KH, 128)      # [dig, kt, p, m2, mc]
        .transpose(3, 1, 2, 0, 4)              # [m2, kt, p, dig, mc]
        .reshape(KH * HIDDEN_DIM, ND * 128)
    ).astype(e5np)
    # who planes padded to 16 output cols: [kt*128, dig*16]
    whod = np.zeros((KH, 128, ND, 16), np.float32)
    whod[:, :, :, :OUTPUT_DIM] = pho.reshape(ND, KH, 128, OUTPUT_DIM) \
        .transpose(1, 2, 0, 3)
    whod8 = np.ascontiguousarray(whod.reshape(KH * 128, ND * 16)).astype(e5np)

    cs_hh = np.ascontiguousarray(
        whh_eff.sum(axis=0, dtype=np.float64).astype(np.float32)
        .reshape(KH, 128).T)
    cs_ho = who_eff.sum(axis=0, dtype=np.float64).astype(np.float32) \
        .reshape(OUTPUT_DIM, 1)

    in_maps = []
    for c in range(NCORES):
        xb = input_bins[c * BLOC:(c + 1) * BLOC]        # [32, 1024, 100]
        xc = np.ascontiguousarray(
            xb.transpose(1, 2, 0).reshape(INPUT_DIM, COLS) * np.float32(MOV)
        ).astype(e5np)
        in_maps.append({
            "x": xc, "wihd": wihd, "whhd": whhd, "whod": whod8,
            "cs_hh": cs_hh, "cs_ho": cs_ho,
            "ident": np.eye(128, dtype=np.float32),
        })

    from concourse.bass_utils import run_bass_kernel_spmd
    res = run_bass_kernel_spmd(nc, in_maps, core_ids=list(range(NCORES)))

    out = np.empty((BATCH, OUTPUT_DIM), dtype=np.float32)
    for c in range(NCORES):
        out[c * BLOC:(c + 1) * BLOC] = res.results[c]["out"].T
    return out
